# revision 23
# baseline (speedup 1.0000x reference)
"""Dense bilinear spatial-transformer warp — 5x5 tri-weight tap window on
device + host patch for outlier flow.

The device evaluates the gatherless tri-weight bilinear warp

    out[y,x] = sum_{dy,dx in [-2,2]} relu(1-|fh-dy|) * relu(1-|fw-dx|) * img[y+dy, x+dx]

which is exact whenever both flow components lie in [-2, 2] (the two bilinear
taps per axis then fall inside the window; taps at the window edge get weight
exactly 0).  For N(0,1) flow that covers ~91% of pixels; the |flow|>2
outliers are patched on the host with the exact clipped-border gather.

Engine split per row-chunk: the Scalar engine builds the per-axis tri-weight
stacks (Abs + Relu activations), the DVE runs the fp16 (2x-mode) tap
multiply/accumulate passes for dy planes 0:4 plus the vertical contraction,
and the GPSIMD/Pool engine independently evaluates the dy=+2 plane, which
balances the two engines' rates.  All three engines plus the DMAs pipeline
across row-chunks (triple-buffered accumulators); the first chunks are
staggered small to shorten pipeline fill.  The image ships as one fp16
zero-padded plane (halo 2); flow ships as raw fp32 row-shards (the
reference's +1 mesh shift folds into the activation biases); the output
returns as fp16 and is upcast on the host.
"""

import time
from contextlib import ExitStack

import numpy as np

import bass_rust
import concourse.bacc as bacc
import concourse.mybir as mybir
import concourse.tile as tile

F32 = mybir.dt.float32
F16 = mybir.dt.float16

H = 4096
W = 4096
NCORES = 8
SH = H // NCORES          # 512 rows per core
HALO = 2                  # tap window [-HALO, HALO] per axis
NTAP = 2 * HALO + 1       # 5
PADW = W + 2 * HALO       # padded image width (4100)
NPART = 128
CPB = W // NPART          # 32 columns per partition
CPB_H = CPB + 2 * HALO    # 36 columns incl. halo
R_CHUNK = 32
SPL = NTAP - 1            # dy planes handled by the DVE (Pool gets the last)


def _band_src_ap(t, row0, r):
    off = row0 * PADW
    return bass_rust.AP(
        tensor=t.ap().tensor, offset=off,
        ap=[[CPB, NPART], [PADW, r + 2 * HALO], [1, CPB_H]],
    )


def _flat_src_ap(t, row0, r, sh_w):
    off = row0 * sh_w
    return bass_rust.AP(
        tensor=t.ap().tensor, offset=off,
        ap=[[CPB, NPART], [sh_w, r], [1, CPB]],
    )


def _stack_view(tile_, width, col_off, nplanes, r):
    """[128, nplanes(dy), r, CPB] view; dy plane j reads rows shifted by j,
    cols shifted by col_off, of a [128, rows, width] tile."""
    base = tile_[:]
    return bass_rust.AP(
        tensor=base.tensor,
        offset=base.offset + col_off,
        ap=[list(base.ap[0]), [width, nplanes], [width, r], [1, CPB]],
    )


def _bcast_planes(ap2d, nplanes):
    return bass_rust.AP(
        tensor=ap2d.tensor, offset=ap2d.offset,
        ap=[list(ap2d.ap[0]), [0, nplanes]] + [list(d) for d in ap2d.ap[1:]],
    )


def _sub(ap, lo, hi):
    """Slice the plane dimension (axis 1) of a 4d AP."""
    return bass_rust.AP(
        tensor=ap.tensor,
        offset=ap.offset + lo * ap.ap[1][0],
        ap=[list(ap.ap[0]), [ap.ap[1][0], hi - lo]]
        + [list(d) for d in ap.ap[2:]],
    )


def build_nc(sh=SH, r_chunk=R_CHUNK, debug=False, head=(8, 8, 16), tail=()):
    nc = bacc.Bacc("TRN2", target_bir_lowering=False, debug=debug)
    # stagger small chunks at both ends to shorten pipeline fill and drain
    head, tail = list(head), list(tail)
    body = (sh - sum(head) - sum(tail)) // r_chunk
    assert sum(head) + sum(tail) + body * r_chunk == sh
    chunks = []
    row0 = 0
    for r in head + [r_chunk] * body + tail:
        chunks.append((row0, r))
        row0 += r

    for v in range(-HALO - 1, HALO + 2):
        val = float(v)
        if (F32, val) not in nc.const_aps.aps:
            t = nc.alloc_sbuf_tensor(f"const-float32-{val}", [128, 1], F32)
            nc.gpsimd.memset(t.ap(), val)
            nc.const_aps.aps[(F32, val)] = t.ap()
    nc.all_engine_barrier()

    img = nc.dram_tensor("img", [sh + 2 * HALO, PADW], F16, kind="ExternalInput")
    fh = nc.dram_tensor("fh", [sh, W], F32, kind="ExternalInput")
    fw = nc.dram_tensor("fw", [sh, W], F32, kind="ExternalInput")
    out = nc.dram_tensor("out", [sh, W], F16, kind="ExternalOutput")

    ABS = mybir.ActivationFunctionType.Abs
    RELU = mybir.ActivationFunctionType.Relu
    MULT = mybir.AluOpType.mult
    ADD = mybir.AluOpType.add

    # (TensorScalarPtr is not a legal Pool-engine opcode on TRN2 silicon,
    # so the GPSIMD side sticks to plain tensor_tensor.)
    def pool_mul(out_ap, a, b):
        nc.gpsimd.tensor_mul(out_ap, a, b)

    def pool_add(out_ap, a, b):
        nc.gpsimd.tensor_add(out_ap, a, b)

    with tile.TileContext(nc) as tc, ExitStack() as ctx:
        io_pool = ctx.enter_context(tc.tile_pool(name="io", bufs=2))
        w_pool = ctx.enter_context(tc.tile_pool(name="wts", bufs=2))
        s_pool = ctx.enter_context(tc.tile_pool(name="stk", bufs=3))
        o_pool = ctx.enter_context(tc.tile_pool(name="out", bufs=2))

        for row0, r in chunks:
            band = io_pool.tile([NPART, r + 2 * HALO, CPB_H], F16, tag="band")
            nc.sync.dma_start(band[:], _band_src_ap(img, row0, r))
            fh_t = io_pool.tile([NPART, r, CPB], F32, tag="fh")
            nc.sync.dma_start(fh_t[:], _flat_src_ap(fh, row0, r, W))
            fw_t = io_pool.tile([NPART, r, CPB], F32, tag="fw")
            nc.sync.dma_start(fw_t[:], _flat_src_ap(fw, row0, r, W))

            # horizontal tri-weight stack relu(1 - |fw - dx|), dx=-2..2
            bstk = w_pool.tile([NPART, NTAP, r, CPB], F16, tag="bstk")
            for i, dx in enumerate(range(-HALO, HALO + 1)):
                nc.scalar.activation(bstk[:, i], fw_t[:], ABS,
                                     bias=float(-dx), scale=1.0)
            nc.scalar.activation(bstk[:], bstk[:], RELU, bias=1.0, scale=-1.0)

            # vertical tri-weight stack relu(1 - |fh - dy|), dy=-2..2
            astk = w_pool.tile([NPART, NTAP, r, CPB], F16, tag="astk")
            for i, dy in enumerate(range(-HALO, HALO + 1)):
                nc.scalar.activation(astk[:, i], fh_t[:], ABS,
                                     bias=float(-dy), scale=1.0)
            nc.scalar.activation(astk[:], astk[:], RELU, bias=1.0, scale=-1.0)

            acc_a = s_pool.tile([NPART, SPL, r, CPB], F16, tag="acc_a")
            tmp = s_pool.tile([NPART, SPL, r, CPB], F16, tag="tmp")
            pacc = s_pool.tile([NPART, 1, r, CPB], F16, tag="pacc")
            ptmp = s_pool.tile([NPART, 1, r, CPB], F16, tag="ptmp")

            bviews = [_stack_view(band, CPB_H, dx + HALO, NTAP, r)
                      for dx in range(-HALO, HALO + 1)]
            cviews = [_bcast_planes(bstk[:, i], NTAP) for i in range(NTAP)]

            # DVE: dy planes 0:4
            nc.vector.tensor_mul(acc_a[:], _sub(cviews[0], 0, SPL),
                                 _sub(bviews[0], 0, SPL))
            for i in range(1, NTAP):
                nc.vector.tensor_mul(tmp[:], _sub(cviews[i], 0, SPL),
                                     _sub(bviews[i], 0, SPL))
                nc.vector.tensor_add(acc_a[:], acc_a[:], tmp[:])

            # Pool: dy plane 4
            pool_mul(pacc[:], _sub(cviews[0], SPL, NTAP),
                     _sub(bviews[0], SPL, NTAP))
            for i in range(1, NTAP):
                pool_mul(ptmp[:], _sub(cviews[i], SPL, NTAP),
                         _sub(bviews[i], SPL, NTAP))
                pool_add(pacc[:], pacc[:], ptmp[:])

            # vertical contraction: the tree add is split one plane per
            # engine; the rest stays on the DVE (Pool results arrive early,
            # so the final +pacc does not stall it)
            nc.vector.tensor_mul(acc_a[:], astk[:, :SPL], acc_a[:])
            pool_mul(pacc[:], astk[:, SPL:], pacc[:])
            out_t = o_pool.tile([NPART, r, CPB], F16, tag="out")
            nc.vector.tensor_add(acc_a[:, :2], acc_a[:, :2], acc_a[:, 2:4])
            nc.vector.tensor_add(out_t[:], acc_a[:, 0], acc_a[:, 1])
            nc.vector.tensor_add(out_t[:], out_t[:], pacc[:, 0])

            nc.sync.dma_start(_flat_src_ap(out, row0, r, W), out_t[:])

    nc.compile()
    return nc


def shard_inputs(input1, input2, sh=SH):
    img = np.asarray(input1, dtype=np.float32).reshape(H, W)
    flow = np.asarray(input2, dtype=np.float32).reshape(2, H, W)
    ncores = H // sh

    img_pad = np.zeros((H + 2 * HALO, PADW), dtype=np.float16)
    img_pad[HALO:H + HALO, HALO:W + HALO] = img

    in_maps = []
    for k in range(ncores):
        h0 = k * sh
        in_maps.append({
            "img": np.ascontiguousarray(img_pad[h0:h0 + sh + 2 * HALO]),
            "fh": np.ascontiguousarray(flow[0, h0:h0 + sh]),
            "fw": np.ascontiguousarray(flow[1, h0:h0 + sh]),
        })
    return in_maps


_NC_CACHE = {}


def _patch_outliers(out, input1, input2):
    """Exact clipped-border bilinear for pixels whose flow leaves the device
    tap window.  Mirrors reference.py's math bit-for-bit in fp32."""
    f32 = np.float32
    flow = np.asarray(input2, dtype=f32).reshape(2, H, W)
    mask = (np.abs(flow[0]) > HALO) | (np.abs(flow[1]) > HALO)
    if not mask.any():
        return out
    img = np.asarray(input1, dtype=f32).reshape(H, W)
    pad = np.zeros((H + 2, W + 2), dtype=f32)
    pad[1:-1, 1:-1] = img
    hy, wx = np.nonzero(mask)
    Hu = (flow[0, hy, wx] + hy.astype(f32)).astype(f32) + f32(1.0)
    Wu = (flow[1, hy, wx] + wx.astype(f32)).astype(f32) + f32(1.0)
    hf = np.floor(Hu).astype(np.int32)
    hc = hf + 1
    wf = np.floor(Wu).astype(np.int32)
    wc = wf + 1
    hfc, hcc = np.clip(hf, 0, H + 1), np.clip(hc, 0, H + 1)
    wfc, wcc = np.clip(wf, 0, W + 1), np.clip(wc, 0, W + 1)
    dH = (hcc.astype(f32) - Hu).astype(f32)
    dW = (wcc.astype(f32) - Wu).astype(f32)
    out[hy, wx] = (
        pad[hfc, wfc] * (dW * dH)
        + pad[hcc, wfc] * (dW * (f32(1.0) - dH))
        + pad[hfc, wcc] * ((f32(1.0) - dW) * dH)
        + pad[hcc, wcc] * ((f32(1.0) - dW) * (f32(1.0) - dH))
    )
    return out


def kernel(input1, input2):
    from concourse.bass_utils import run_bass_kernel_spmd

    in_maps = shard_inputs(input1, input2)
    key = (SH, R_CHUNK, HALO)
    if key not in _NC_CACHE:
        _NC_CACHE[key] = build_nc(sh=SH, r_chunk=R_CHUNK)
    nc = _NC_CACHE[key]

    last_err = None
    for attempt in range(3):
        try:
            res = run_bass_kernel_spmd(nc, in_maps, core_ids=list(range(NCORES)))
            break
        except Exception as e:  # transient device desync — retry
            last_err = e
            time.sleep(5.0 * (attempt + 1))
    else:
        raise last_err
    out = np.concatenate([r["out"] for r in res.results], axis=0).astype(np.float32)

    out = _patch_outliers(out, input1, input2)
    return out.reshape(1, 1, H, W)


# revision 34
# speedup vs baseline: 1.0090x; 1.0090x over previous
"""Dense bilinear spatial-transformer warp — 5x5 tri-weight tap window on
device + host patch for outlier flow.

The device evaluates the gatherless tri-weight bilinear warp

    out[y,x] = sum_{dy,dx in [-2,2]} relu(1-|fh-dy|) * relu(1-|fw-dx|) * img[y+dy, x+dx]

which is exact whenever both flow components lie in [-2, 2] (the two bilinear
taps per axis then fall inside the window; taps at the window edge get weight
exactly 0).  For N(0,1) flow that covers ~91% of pixels; the |flow|>2
outliers are patched on the host with the exact clipped-border gather.

Engine split per row-chunk: the Scalar engine builds the per-axis tri-weight
stacks (Abs + Relu activations), the DVE runs the fp16 (2x-mode) tap
multiply/accumulate passes for dy planes 0:4 plus the vertical contraction,
and the GPSIMD/Pool engine independently evaluates the dy=+2 plane, which
balances the two engines' rates.  All three engines plus the DMAs pipeline
across row-chunks (triple-buffered accumulators); the first chunks are
staggered small to shorten pipeline fill.  The image ships as one fp16
zero-padded plane (halo 2); flow ships as raw fp32 row-shards (the
reference's +1 mesh shift folds into the activation biases); the output
returns as fp16 and is upcast on the host.
"""

import time
from contextlib import ExitStack

import numpy as np

import bass_rust
import concourse.bacc as bacc
import concourse.mybir as mybir
import concourse.tile as tile

F32 = mybir.dt.float32
F16 = mybir.dt.float16

H = 4096
W = 4096
NCORES = 8
SH = H // NCORES          # 512 rows per core
HALO = 2                  # tap window [-HALO, HALO] per axis
NTAP = 2 * HALO + 1       # 5
PADW = W + 2 * HALO       # padded image width (4100)
NPART = 128
CPB = W // NPART          # 32 columns per partition
CPB_H = CPB + 2 * HALO    # 36 columns incl. halo
R_CHUNK = 48
SPL = NTAP - 1            # dy planes handled by the DVE (Pool gets the last)


def _band_src_ap(t, row0, r):
    off = row0 * PADW
    return bass_rust.AP(
        tensor=t.ap().tensor, offset=off,
        ap=[[CPB, NPART], [PADW, r + 2 * HALO], [1, CPB_H]],
    )


def _flat_src_ap(t, row0, r, sh_w):
    off = row0 * sh_w
    return bass_rust.AP(
        tensor=t.ap().tensor, offset=off,
        ap=[[CPB, NPART], [sh_w, r], [1, CPB]],
    )


def _stack_view(tile_, width, col_off, nplanes, r):
    """[128, nplanes(dy), r, CPB] view; dy plane j reads rows shifted by j,
    cols shifted by col_off, of a [128, rows, width] tile."""
    base = tile_[:]
    return bass_rust.AP(
        tensor=base.tensor,
        offset=base.offset + col_off,
        ap=[list(base.ap[0]), [width, nplanes], [width, r], [1, CPB]],
    )


def _bcast_planes(ap2d, nplanes):
    return bass_rust.AP(
        tensor=ap2d.tensor, offset=ap2d.offset,
        ap=[list(ap2d.ap[0]), [0, nplanes]] + [list(d) for d in ap2d.ap[1:]],
    )


def _sub(ap, lo, hi):
    """Slice the plane dimension (axis 1) of a 4d AP."""
    return bass_rust.AP(
        tensor=ap.tensor,
        offset=ap.offset + lo * ap.ap[1][0],
        ap=[list(ap.ap[0]), [ap.ap[1][0], hi - lo]]
        + [list(d) for d in ap.ap[2:]],
    )


def build_nc(sh=SH, r_chunk=R_CHUNK, debug=False, head=(8, 8, 16), tail=(),
             out2_pool=False, out3_pool=True, stk_bufs=2, dma_merge=False):
    nc = bacc.Bacc("TRN2", target_bir_lowering=False, debug=debug)
    # stagger small chunks at both ends to shorten pipeline fill and drain
    head, tail = list(head), list(tail)
    body = (sh - sum(head) - sum(tail)) // r_chunk
    assert sum(head) + sum(tail) + body * r_chunk == sh
    chunks = []
    row0 = 0
    for r in head + [r_chunk] * body + tail:
        chunks.append((row0, r))
        row0 += r

    for v in range(-HALO - 1, HALO + 2):
        val = float(v)
        if (F32, val) not in nc.const_aps.aps:
            t = nc.alloc_sbuf_tensor(f"const-float32-{val}", [128, 1], F32)
            nc.gpsimd.memset(t.ap(), val)
            nc.const_aps.aps[(F32, val)] = t.ap()
    nc.all_engine_barrier()

    img = nc.dram_tensor("img", [sh + 2 * HALO, PADW], F16, kind="ExternalInput")
    fh = nc.dram_tensor("fh", [sh, W], F32, kind="ExternalInput")
    fw = nc.dram_tensor("fw", [sh, W], F32, kind="ExternalInput")
    out = nc.dram_tensor("out", [sh, W], F16, kind="ExternalOutput")

    ABS = mybir.ActivationFunctionType.Abs
    RELU = mybir.ActivationFunctionType.Relu
    MULT = mybir.AluOpType.mult
    ADD = mybir.AluOpType.add

    # (TensorScalarPtr is not a legal Pool-engine opcode on TRN2 silicon,
    # so the GPSIMD side sticks to plain tensor_tensor.)
    def pool_mul(out_ap, a, b):
        nc.gpsimd.tensor_mul(out_ap, a, b)

    def pool_add(out_ap, a, b):
        nc.gpsimd.tensor_add(out_ap, a, b)

    with tile.TileContext(nc) as tc, ExitStack() as ctx:
        io_pool = ctx.enter_context(tc.tile_pool(name="io", bufs=2))
        w_pool = ctx.enter_context(tc.tile_pool(name="wts", bufs=2))
        s_pool = ctx.enter_context(tc.tile_pool(name="stk", bufs=stk_bufs))
        o_pool = ctx.enter_context(tc.tile_pool(name="out", bufs=2))

        pending = [None]

        def emit_vert(row0, r, acc_a, pacc, astk):
            # vertical contraction for an earlier chunk (deferred so the
            # accumulator-merge DMA latency hides behind the next chunk's
            # tap passes when dma_merge is on)
            nc.vector.tensor_mul(acc_a[:], astk[:, :SPL], acc_a[:])
            pool_mul(pacc[:], astk[:, SPL:], pacc[:])
            out_t = o_pool.tile([NPART, r, CPB], F16, tag="out")
            nc.vector.tensor_add(acc_a[:, :2], acc_a[:, :2], acc_a[:, 2:4])
            if out2_pool:
                pool_add(out_t[:], acc_a[:, 0], acc_a[:, 1])
            else:
                nc.vector.tensor_add(out_t[:], acc_a[:, 0], acc_a[:, 1])
            if out3_pool:
                pool_add(out_t[:], out_t[:], pacc[:, 0])
            else:
                nc.vector.tensor_add(out_t[:], out_t[:], pacc[:, 0])
            nc.sync.dma_start(_flat_src_ap(out, row0, r, W), out_t[:])

        for row0, r in chunks:
            band = io_pool.tile([NPART, r + 2 * HALO, CPB_H], F16, tag="band")
            nc.sync.dma_start(band[:], _band_src_ap(img, row0, r))
            fh_t = io_pool.tile([NPART, r, CPB], F32, tag="fh")
            nc.sync.dma_start(fh_t[:], _flat_src_ap(fh, row0, r, W))
            fw_t = io_pool.tile([NPART, r, CPB], F32, tag="fw")
            nc.sync.dma_start(fw_t[:], _flat_src_ap(fw, row0, r, W))

            # horizontal tri-weight stack relu(1 - |fw - dx|), dx=-2..2
            bstk = w_pool.tile([NPART, NTAP, r, CPB], F16, tag="bstk")
            for i, dx in enumerate(range(-HALO, HALO + 1)):
                nc.scalar.activation(bstk[:, i], fw_t[:], ABS,
                                     bias=float(-dx), scale=1.0)
            nc.scalar.activation(bstk[:], bstk[:], RELU, bias=1.0, scale=-1.0)

            # vertical tri-weight stack relu(1 - |fh - dy|), dy=-2..2
            astk = w_pool.tile([NPART, NTAP, r, CPB], F16, tag="astk")
            for i, dy in enumerate(range(-HALO, HALO + 1)):
                nc.scalar.activation(astk[:, i], fh_t[:], ABS,
                                     bias=float(-dy), scale=1.0)
            nc.scalar.activation(astk[:], astk[:], RELU, bias=1.0, scale=-1.0)

            acc_a = s_pool.tile([NPART, SPL, r, CPB], F16, tag="acc_a")
            tmp = s_pool.tile([NPART, SPL, r, CPB], F16, tag="tmp")
            pacc = s_pool.tile([NPART, 1, r, CPB], F16, tag="pacc")
            ptmp = s_pool.tile([NPART, 1, r, CPB], F16, tag="ptmp")
            if dma_merge:
                acc_b = s_pool.tile([NPART, SPL, r, CPB], F16, tag="acc_b")
            else:
                acc_b = None

            bviews = [_stack_view(band, CPB_H, dx + HALO, NTAP, r)
                      for dx in range(-HALO, HALO + 1)]
            cviews = [_bcast_planes(bstk[:, i], NTAP) for i in range(NTAP)]

            # DVE: dy planes 0:4
            if dma_merge:
                nc.vector.tensor_mul(acc_a[:], _sub(cviews[0], 0, SPL),
                                     _sub(bviews[0], 0, SPL))
                for i in (1, 2):
                    nc.vector.tensor_mul(tmp[:], _sub(cviews[i], 0, SPL),
                                         _sub(bviews[i], 0, SPL))
                    nc.vector.tensor_add(acc_a[:], acc_a[:], tmp[:])
                    if i == 1 and pending[0] is not None:
                        emit_vert(*pending[0])
                        pending[0] = None
                nc.vector.tensor_mul(acc_b[:], _sub(cviews[3], 0, SPL),
                                     _sub(bviews[3], 0, SPL))
                nc.vector.tensor_mul(tmp[:], _sub(cviews[4], 0, SPL),
                                     _sub(bviews[4], 0, SPL))
                nc.vector.tensor_add(acc_b[:], acc_b[:], tmp[:])
            else:
                nc.vector.tensor_mul(acc_a[:], _sub(cviews[0], 0, SPL),
                                     _sub(bviews[0], 0, SPL))
                for i in range(1, NTAP):
                    nc.vector.tensor_mul(tmp[:], _sub(cviews[i], 0, SPL),
                                         _sub(bviews[i], 0, SPL))
                    nc.vector.tensor_add(acc_a[:], acc_a[:], tmp[:])
                    if i == 1 and pending[0] is not None:
                        emit_vert(*pending[0])
                        pending[0] = None

            # Pool: dy plane 4
            pool_mul(pacc[:], _sub(cviews[0], SPL, NTAP),
                     _sub(bviews[0], SPL, NTAP))
            for i in range(1, NTAP):
                pool_mul(ptmp[:], _sub(cviews[i], SPL, NTAP),
                         _sub(bviews[i], SPL, NTAP))
                pool_add(pacc[:], pacc[:], ptmp[:])
            if dma_merge:
                # issued after Pool's own taps so the in-order Pool queue
                # reaches the descriptor-gen with its operands already ready;
                # flat per-partition APs so each partition is one descriptor
                def _flat(t):
                    b = t[:]
                    return bass_rust.AP(tensor=b.tensor, offset=b.offset,
                                        ap=[list(b.ap[0]), [1, SPL * r * CPB]])
                nc.gpsimd.dma_start(_flat(acc_a), _flat(acc_b), accum_op=ADD)

            pending[0] = (row0, r, acc_a, pacc, astk)

        emit_vert(*pending[0])

    nc.compile()
    return nc


def shard_inputs(input1, input2, sh=SH):
    img = np.asarray(input1, dtype=np.float32).reshape(H, W)
    flow = np.asarray(input2, dtype=np.float32).reshape(2, H, W)
    ncores = H // sh

    img_pad = np.zeros((H + 2 * HALO, PADW), dtype=np.float16)
    img_pad[HALO:H + HALO, HALO:W + HALO] = img

    in_maps = []
    for k in range(ncores):
        h0 = k * sh
        in_maps.append({
            "img": np.ascontiguousarray(img_pad[h0:h0 + sh + 2 * HALO]),
            "fh": np.ascontiguousarray(flow[0, h0:h0 + sh]),
            "fw": np.ascontiguousarray(flow[1, h0:h0 + sh]),
        })
    return in_maps


_NC_CACHE = {}


def _patch_outliers(out, input1, input2):
    """Exact clipped-border bilinear for pixels whose flow leaves the device
    tap window.  Mirrors reference.py's math bit-for-bit in fp32."""
    f32 = np.float32
    flow = np.asarray(input2, dtype=f32).reshape(2, H, W)
    mask = (np.abs(flow[0]) > HALO) | (np.abs(flow[1]) > HALO)
    if not mask.any():
        return out
    img = np.asarray(input1, dtype=f32).reshape(H, W)
    pad = np.zeros((H + 2, W + 2), dtype=f32)
    pad[1:-1, 1:-1] = img
    hy, wx = np.nonzero(mask)
    Hu = (flow[0, hy, wx] + hy.astype(f32)).astype(f32) + f32(1.0)
    Wu = (flow[1, hy, wx] + wx.astype(f32)).astype(f32) + f32(1.0)
    hf = np.floor(Hu).astype(np.int32)
    hc = hf + 1
    wf = np.floor(Wu).astype(np.int32)
    wc = wf + 1
    hfc, hcc = np.clip(hf, 0, H + 1), np.clip(hc, 0, H + 1)
    wfc, wcc = np.clip(wf, 0, W + 1), np.clip(wc, 0, W + 1)
    dH = (hcc.astype(f32) - Hu).astype(f32)
    dW = (wcc.astype(f32) - Wu).astype(f32)
    out[hy, wx] = (
        pad[hfc, wfc] * (dW * dH)
        + pad[hcc, wfc] * (dW * (f32(1.0) - dH))
        + pad[hfc, wcc] * ((f32(1.0) - dW) * dH)
        + pad[hcc, wcc] * ((f32(1.0) - dW) * (f32(1.0) - dH))
    )
    return out


def kernel(input1, input2):
    from concourse.bass_utils import run_bass_kernel_spmd

    in_maps = shard_inputs(input1, input2)
    key = (SH, R_CHUNK, HALO)
    if key not in _NC_CACHE:
        _NC_CACHE[key] = build_nc(sh=SH, r_chunk=R_CHUNK)
    nc = _NC_CACHE[key]

    last_err = None
    for attempt in range(3):
        try:
            res = run_bass_kernel_spmd(nc, in_maps, core_ids=list(range(NCORES)))
            break
        except Exception as e:  # transient device desync — retry
            last_err = e
            time.sleep(5.0 * (attempt + 1))
    else:
        raise last_err
    out = np.concatenate([r["out"] for r in res.results], axis=0).astype(np.float32)

    out = _patch_outliers(out, input1, input2)
    return out.reshape(1, 1, H, W)


# revision 43
# speedup vs baseline: 1.0511x; 1.0417x over previous
"""Dense bilinear spatial-transformer warp — 5x5 tri-weight tap window on
device + host patch for outlier flow.

The device evaluates the gatherless tri-weight bilinear warp

    out[y,x] = sum_{dy,dx in [-2,2]} relu(1-|fh-dy|) * relu(1-|fw-dx|) * img[y+dy, x+dx]

which is exact whenever both flow components lie in [-2, 2] (the two bilinear
taps per axis then fall inside the window; taps at the window edge get weight
exactly 0).  For N(0,1) flow that covers ~91% of pixels; the |flow|>2
outliers are patched on the host with the exact clipped-border gather.

Engine split per row-chunk: the Scalar engine builds the per-axis tri-weight
stacks (Abs + Relu activations), the DVE runs the fp16 (2x-mode) tap
multiply/accumulate passes for dy planes 0:4 plus the vertical contraction,
and the GPSIMD/Pool engine independently evaluates the dy=+2 plane, which
balances the two engines' rates.  All three engines plus the DMAs pipeline
across row-chunks (triple-buffered accumulators); the first chunks are
staggered small to shorten pipeline fill.  The image ships as one fp16
zero-padded plane (halo 2); flow ships as raw fp32 row-shards (the
reference's +1 mesh shift folds into the activation biases); the output
returns as fp16 and is upcast on the host.
"""

import time
from contextlib import ExitStack

import numpy as np

import bass_rust
import concourse.bacc as bacc
import concourse.mybir as mybir
import concourse.tile as tile

F32 = mybir.dt.float32
F16 = mybir.dt.float16

H = 4096
W = 4096
NCORES = 8
SH = H // NCORES          # 512 rows per core
HALO = 2                  # tap window [-HALO, HALO] per axis
NTAP = 2 * HALO + 1       # 5
PADW = W + 2 * HALO       # padded image width (4100)
NPART = 128
CPB = W // NPART          # 32 columns per partition
CPB_H = CPB + 2 * HALO    # 36 columns incl. halo
R_CHUNK = 48
SPL = NTAP - 1            # dy planes handled by the DVE (Pool gets the last)


def _band_src_ap(t, row0, r):
    off = row0 * PADW
    return bass_rust.AP(
        tensor=t.ap().tensor, offset=off,
        ap=[[CPB, NPART], [PADW, r + 2 * HALO], [1, CPB_H]],
    )


def _flat_src_ap(t, row0, r, sh_w):
    off = row0 * sh_w
    return bass_rust.AP(
        tensor=t.ap().tensor, offset=off,
        ap=[[CPB, NPART], [sh_w, r], [1, CPB]],
    )


def _stack_view(tile_, width, col_off, nplanes, r):
    """[128, nplanes(dy), r, CPB] view; dy plane j reads rows shifted by j,
    cols shifted by col_off, of a [128, rows, width] tile."""
    base = tile_[:]
    return bass_rust.AP(
        tensor=base.tensor,
        offset=base.offset + col_off,
        ap=[list(base.ap[0]), [width, nplanes], [width, r], [1, CPB]],
    )


def _bcast_planes(ap2d, nplanes):
    return bass_rust.AP(
        tensor=ap2d.tensor, offset=ap2d.offset,
        ap=[list(ap2d.ap[0]), [0, nplanes]] + [list(d) for d in ap2d.ap[1:]],
    )


def _sub(ap, lo, hi):
    """Slice the plane dimension (axis 1) of a 4d AP."""
    return bass_rust.AP(
        tensor=ap.tensor,
        offset=ap.offset + lo * ap.ap[1][0],
        ap=[list(ap.ap[0]), [ap.ap[1][0], hi - lo]]
        + [list(d) for d in ap.ap[2:]],
    )


def build_nc(sh=SH, r_chunk=R_CHUNK, debug=False, head=(8, 8, 16), tail=(),
             out2_pool=False, out3_pool=True, stk_bufs=2, dma_merge=False,
             io_bufs=2, w_bufs=2, tele=True):
    nc = bacc.Bacc("TRN2", target_bir_lowering=False, debug=debug)
    # stagger small chunks at both ends to shorten pipeline fill and drain
    head, tail = list(head), list(tail)
    body = (sh - sum(head) - sum(tail)) // r_chunk
    assert sum(head) + sum(tail) + body * r_chunk == sh
    chunks = []
    row0 = 0
    for r in head + [r_chunk] * body + tail:
        chunks.append((row0, r))
        row0 += r

    for v in range(-HALO - 1, HALO + 2):
        val = float(v)
        if (F32, val) not in nc.const_aps.aps:
            t = nc.alloc_sbuf_tensor(f"const-float32-{val}", [128, 1], F32)
            nc.gpsimd.memset(t.ap(), val)
            nc.const_aps.aps[(F32, val)] = t.ap()
    nc.all_engine_barrier()

    img = nc.dram_tensor("img", [sh + 2 * HALO, PADW], F16, kind="ExternalInput")
    fh = nc.dram_tensor("fh", [sh, W], F32, kind="ExternalInput")
    fw = nc.dram_tensor("fw", [sh, W], F32, kind="ExternalInput")
    out = nc.dram_tensor("out", [sh, W], F16, kind="ExternalOutput")

    ABS = mybir.ActivationFunctionType.Abs
    RELU = mybir.ActivationFunctionType.Relu
    MULT = mybir.AluOpType.mult
    ADD = mybir.AluOpType.add

    # (TensorScalarPtr is not a legal Pool-engine opcode on TRN2 silicon,
    # so the GPSIMD side sticks to plain tensor_tensor.)
    def pool_mul(out_ap, a, b):
        nc.gpsimd.tensor_mul(out_ap, a, b)

    def pool_add(out_ap, a, b):
        nc.gpsimd.tensor_add(out_ap, a, b)

    with tile.TileContext(nc) as tc, ExitStack() as ctx:
        io_pool = ctx.enter_context(tc.tile_pool(name="io", bufs=io_bufs))
        w_pool = ctx.enter_context(tc.tile_pool(name="wts", bufs=w_bufs))
        s_pool = ctx.enter_context(tc.tile_pool(name="stk", bufs=stk_bufs))
        o_pool = ctx.enter_context(tc.tile_pool(name="out", bufs=2))

        pending = [None]

        def emit_vert(row0, r, acc_a, pacc, astk):
            # vertical contraction for an earlier chunk (deferred so the
            # accumulator-merge DMA latency hides behind the next chunk's
            # tap passes when dma_merge is on)
            nc.vector.tensor_mul(acc_a[:], astk[:, :SPL], acc_a[:])
            pool_mul(pacc[:], astk[:, SPL:], pacc[:])
            out_t = o_pool.tile([NPART, r, CPB], F16, tag="out")
            nc.vector.tensor_add(acc_a[:, :2], acc_a[:, :2], acc_a[:, 2:4])
            if out2_pool:
                pool_add(out_t[:], acc_a[:, 0], acc_a[:, 1])
            else:
                nc.vector.tensor_add(out_t[:], acc_a[:, 0], acc_a[:, 1])
            if out3_pool:
                pool_add(out_t[:], out_t[:], pacc[:, 0])
            else:
                nc.vector.tensor_add(out_t[:], out_t[:], pacc[:, 0])
            nc.sync.dma_start(_flat_src_ap(out, row0, r, W), out_t[:])

        for row0, r in chunks:
            band = io_pool.tile([NPART, r + 2 * HALO, CPB_H], F16, tag="band")
            nc.sync.dma_start(band[:], _band_src_ap(img, row0, r))
            fh_t = io_pool.tile([NPART, r, CPB], F32, tag="fh")
            nc.sync.dma_start(fh_t[:], _flat_src_ap(fh, row0, r, W))
            fw_t = io_pool.tile([NPART, r, CPB], F32, tag="fw")
            nc.sync.dma_start(fw_t[:], _flat_src_ap(fw, row0, r, W))

            if tele:
                # telescoped horizontal: difference band D[j,c] = B[j,c+1] -
                # B[j,c] plus clamp weights c(dx) = clamp(fw - dx, 0, 1):
                #   H = B[x-2] + sum_{dx=-2..1} D[x+dx] * c(dx)
                # (exact piecewise-linear interpolation for fw in [-2, 2];
                # one fewer tap pass per engine than the tri-weight form)
                dband = io_pool.tile([NPART, r + 2 * HALO, CPB_H - 1], F16,
                                     tag="dband")
                bb = band[:]
                b_lo = bass_rust.AP(
                    tensor=bb.tensor, offset=bb.offset,
                    ap=[list(bb.ap[0]), [CPB_H, r + 2 * HALO],
                        [1, CPB_H - 1]])
                b_hi = bass_rust.AP(
                    tensor=bb.tensor, offset=bb.offset + 1,
                    ap=[list(bb.ap[0]), [CPB_H, r + 2 * HALO],
                        [1, CPB_H - 1]])
                nc.vector.tensor_sub(dband[:], b_hi, b_lo)

                bstk = w_pool.tile([NPART, NTAP - 1, r, CPB], F16, tag="bstk")
                for i, dx in enumerate(range(-HALO, HALO)):
                    nc.scalar.activation(bstk[:, i], fw_t[:], RELU,
                                         bias=float(-dx), scale=1.0)
                # clamp upper bound; tensor_scalar runs in DVE 4x mode
                nc.vector.tensor_scalar_min(bstk[:], bstk[:], 1.0)
            else:
                # horizontal tri-weight stack relu(1 - |fw - dx|), dx=-2..2
                bstk = w_pool.tile([NPART, NTAP, r, CPB], F16, tag="bstk")
                for i, dx in enumerate(range(-HALO, HALO + 1)):
                    nc.scalar.activation(bstk[:, i], fw_t[:], ABS,
                                         bias=float(-dx), scale=1.0)
                nc.scalar.activation(bstk[:], bstk[:], RELU,
                                     bias=1.0, scale=-1.0)

            # vertical tri-weight stack relu(1 - |fh - dy|), dy=-2..2
            astk = w_pool.tile([NPART, NTAP, r, CPB], F16, tag="astk")
            for i, dy in enumerate(range(-HALO, HALO + 1)):
                nc.scalar.activation(astk[:, i], fh_t[:], ABS,
                                     bias=float(-dy), scale=1.0)
            nc.scalar.activation(astk[:], astk[:], RELU, bias=1.0, scale=-1.0)

            acc_a = s_pool.tile([NPART, SPL, r, CPB], F16, tag="acc_a")
            tmp = s_pool.tile([NPART, SPL, r, CPB], F16, tag="tmp")
            pacc = s_pool.tile([NPART, 1, r, CPB], F16, tag="pacc")
            ptmp = s_pool.tile([NPART, 1, r, CPB], F16, tag="ptmp")

            if tele:
                ntx = NTAP - 1
                bviews = [_stack_view(dband, CPB_H - 1, dx + HALO, NTAP, r)
                          for dx in range(-HALO, HALO)]
                basev = _stack_view(band, CPB_H, 0, NTAP, r)
            else:
                ntx = NTAP
                bviews = [_stack_view(band, CPB_H, dx + HALO, NTAP, r)
                          for dx in range(-HALO, HALO + 1)]
                basev = None
            cviews = [_bcast_planes(bstk[:, i], NTAP) for i in range(ntx)]

            # DVE: dy planes 0:4
            nc.vector.tensor_mul(acc_a[:], _sub(cviews[0], 0, SPL),
                                 _sub(bviews[0], 0, SPL))
            for i in range(1, ntx):
                nc.vector.tensor_mul(tmp[:], _sub(cviews[i], 0, SPL),
                                     _sub(bviews[i], 0, SPL))
                nc.vector.tensor_add(acc_a[:], acc_a[:], tmp[:])
                if i == 1 and pending[0] is not None:
                    emit_vert(*pending[0])
                    pending[0] = None
            if tele:
                nc.vector.tensor_add(acc_a[:], acc_a[:], _sub(basev, 0, SPL))

            # Pool: dy plane 4
            pool_mul(pacc[:], _sub(cviews[0], SPL, NTAP),
                     _sub(bviews[0], SPL, NTAP))
            for i in range(1, ntx):
                pool_mul(ptmp[:], _sub(cviews[i], SPL, NTAP),
                         _sub(bviews[i], SPL, NTAP))
                pool_add(pacc[:], pacc[:], ptmp[:])
            if tele:
                pool_add(pacc[:], pacc[:], _sub(basev, SPL, NTAP))

            pending[0] = (row0, r, acc_a, pacc, astk)

        emit_vert(*pending[0])

    nc.compile()
    return nc


def shard_inputs(input1, input2, sh=SH):
    img = np.asarray(input1, dtype=np.float32).reshape(H, W)
    flow = np.asarray(input2, dtype=np.float32).reshape(2, H, W)
    ncores = H // sh

    img_pad = np.zeros((H + 2 * HALO, PADW), dtype=np.float16)
    img_pad[HALO:H + HALO, HALO:W + HALO] = img

    in_maps = []
    for k in range(ncores):
        h0 = k * sh
        in_maps.append({
            "img": np.ascontiguousarray(img_pad[h0:h0 + sh + 2 * HALO]),
            "fh": np.ascontiguousarray(flow[0, h0:h0 + sh]),
            "fw": np.ascontiguousarray(flow[1, h0:h0 + sh]),
        })
    return in_maps


_NC_CACHE = {}


def _patch_outliers(out, input1, input2):
    """Exact clipped-border bilinear for pixels whose flow leaves the device
    tap window.  Mirrors reference.py's math bit-for-bit in fp32."""
    f32 = np.float32
    flow = np.asarray(input2, dtype=f32).reshape(2, H, W)
    mask = (np.abs(flow[0]) > HALO) | (np.abs(flow[1]) > HALO)
    if not mask.any():
        return out
    img = np.asarray(input1, dtype=f32).reshape(H, W)
    pad = np.zeros((H + 2, W + 2), dtype=f32)
    pad[1:-1, 1:-1] = img
    hy, wx = np.nonzero(mask)
    Hu = (flow[0, hy, wx] + hy.astype(f32)).astype(f32) + f32(1.0)
    Wu = (flow[1, hy, wx] + wx.astype(f32)).astype(f32) + f32(1.0)
    hf = np.floor(Hu).astype(np.int32)
    hc = hf + 1
    wf = np.floor(Wu).astype(np.int32)
    wc = wf + 1
    hfc, hcc = np.clip(hf, 0, H + 1), np.clip(hc, 0, H + 1)
    wfc, wcc = np.clip(wf, 0, W + 1), np.clip(wc, 0, W + 1)
    dH = (hcc.astype(f32) - Hu).astype(f32)
    dW = (wcc.astype(f32) - Wu).astype(f32)
    out[hy, wx] = (
        pad[hfc, wfc] * (dW * dH)
        + pad[hcc, wfc] * (dW * (f32(1.0) - dH))
        + pad[hfc, wcc] * ((f32(1.0) - dW) * dH)
        + pad[hcc, wcc] * ((f32(1.0) - dW) * (f32(1.0) - dH))
    )
    return out


def kernel(input1, input2):
    from concourse.bass_utils import run_bass_kernel_spmd

    in_maps = shard_inputs(input1, input2)
    key = (SH, R_CHUNK, HALO)
    if key not in _NC_CACHE:
        _NC_CACHE[key] = build_nc(sh=SH, r_chunk=R_CHUNK)
    nc = _NC_CACHE[key]

    last_err = None
    for attempt in range(3):
        try:
            res = run_bass_kernel_spmd(nc, in_maps, core_ids=list(range(NCORES)))
            break
        except Exception as e:  # transient device desync — retry
            last_err = e
            time.sleep(5.0 * (attempt + 1))
    else:
        raise last_err
    out = np.concatenate([r["out"] for r in res.results], axis=0).astype(np.float32)

    out = _patch_outliers(out, input1, input2)
    return out.reshape(1, 1, H, W)


# revision 47
# speedup vs baseline: 1.0699x; 1.0178x over previous
"""Dense bilinear spatial-transformer warp — gatherless 5-row tap window on
device + host patch for outlier flow.

Device math, exact whenever both flow components lie in [-2, 2] (for N(0,1)
flow that covers ~91% of pixels; the |flow|>2 outliers are patched on the
host with the exact clipped-border gather):

  horizontal, per dy row (telescoped interpolation — 4 difference taps+base):
      H[dy] = B[y+dy, x-2] + sum_{dx=-2..1} D[y+dy, x+dx] * clamp(fw-dx, 0, 1)
      with the difference band D[j, c] = B[j, c+1] - B[j, c]
  vertical (tri-weight, taps at the window edge get weight exactly 0):
      out   = sum_{dy=-2..2} relu(1 - |fh - dy|) * H[dy]

Engine split per row-chunk: the Scalar engine builds the per-axis tri-weight
stacks (Abs + Relu activations), the DVE runs the fp16 (2x-mode) tap
multiply/accumulate passes for dy planes 0:4 plus the vertical contraction,
and the GPSIMD/Pool engine independently evaluates the dy=+2 plane, which
balances the two engines' rates.  All three engines plus the DMAs pipeline
across row-chunks (triple-buffered accumulators); the first chunks are
staggered small to shorten pipeline fill.  The image ships as one fp16
zero-padded plane (halo 2); flow ships as raw fp32 row-shards (the
reference's +1 mesh shift folds into the activation biases); the output
returns as fp16 and is upcast on the host.
"""

import time
from contextlib import ExitStack

import numpy as np

import bass_rust
import concourse.bacc as bacc
import concourse.mybir as mybir
import concourse.tile as tile

F32 = mybir.dt.float32
F16 = mybir.dt.float16

H = 4096
W = 4096
NCORES = 8
SH = H // NCORES          # 512 rows per core
HALO = 2                  # tap window [-HALO, HALO] per axis
NTAP = 2 * HALO + 1       # 5
PADW = W + 2 * HALO       # padded image width (4100)
NPART = 128
CPB = W // NPART          # 32 columns per partition
CPB_H = CPB + 2 * HALO    # 36 columns incl. halo
R_CHUNK = 48
SPL = NTAP - 1            # dy planes handled by the DVE (Pool gets the last)


def _band_src_ap(t, row0, r):
    off = row0 * PADW
    return bass_rust.AP(
        tensor=t.ap().tensor, offset=off,
        ap=[[CPB, NPART], [PADW, r + 2 * HALO], [1, CPB_H]],
    )


def _flat_src_ap(t, row0, r, sh_w):
    off = row0 * sh_w
    return bass_rust.AP(
        tensor=t.ap().tensor, offset=off,
        ap=[[CPB, NPART], [sh_w, r], [1, CPB]],
    )


def _stack_view(tile_, width, col_off, nplanes, r):
    """[128, nplanes(dy), r, CPB] view; dy plane j reads rows shifted by j,
    cols shifted by col_off, of a [128, rows, width] tile."""
    base = tile_[:]
    return bass_rust.AP(
        tensor=base.tensor,
        offset=base.offset + col_off,
        ap=[list(base.ap[0]), [width, nplanes], [width, r], [1, CPB]],
    )


def _bcast_planes(ap2d, nplanes):
    return bass_rust.AP(
        tensor=ap2d.tensor, offset=ap2d.offset,
        ap=[list(ap2d.ap[0]), [0, nplanes]] + [list(d) for d in ap2d.ap[1:]],
    )


def _sub(ap, lo, hi):
    """Slice the plane dimension (axis 1) of a 4d AP."""
    return bass_rust.AP(
        tensor=ap.tensor,
        offset=ap.offset + lo * ap.ap[1][0],
        ap=[list(ap.ap[0]), [ap.ap[1][0], hi - lo]]
        + [list(d) for d in ap.ap[2:]],
    )


def build_nc(sh=SH, r_chunk=R_CHUNK, debug=False, head=(8, 8, 16), tail=(),
             out2_pool=False, out3_pool=True, stk_bufs=2, dma_merge=False,
             io_bufs=2, w_bufs=2, tele=True):
    nc = bacc.Bacc("TRN2", target_bir_lowering=False, debug=debug)
    # stagger small chunks at both ends to shorten pipeline fill and drain
    head, tail = list(head), list(tail)
    body = (sh - sum(head) - sum(tail)) // r_chunk
    assert sum(head) + sum(tail) + body * r_chunk == sh
    chunks = []
    row0 = 0
    for r in head + [r_chunk] * body + tail:
        chunks.append((row0, r))
        row0 += r

    for v in range(-HALO - 1, HALO + 2):
        val = float(v)
        if (F32, val) not in nc.const_aps.aps:
            t = nc.alloc_sbuf_tensor(f"const-float32-{val}", [128, 1], F32)
            nc.gpsimd.memset(t.ap(), val)
            nc.const_aps.aps[(F32, val)] = t.ap()
    nc.all_engine_barrier()

    img = nc.dram_tensor("img", [sh + 2 * HALO, PADW], F16, kind="ExternalInput")
    dimg = nc.dram_tensor("dimg", [sh + 2 * HALO, PADW - 1], F16,
                          kind="ExternalInput")
    fh = nc.dram_tensor("fh", [sh, W], F32, kind="ExternalInput")
    fw = nc.dram_tensor("fw", [sh, W], F32, kind="ExternalInput")
    out = nc.dram_tensor("out", [sh, W], F16, kind="ExternalOutput")

    ABS = mybir.ActivationFunctionType.Abs
    RELU = mybir.ActivationFunctionType.Relu
    MULT = mybir.AluOpType.mult
    ADD = mybir.AluOpType.add

    # (TensorScalarPtr is not a legal Pool-engine opcode on TRN2 silicon,
    # so the GPSIMD side sticks to plain tensor_tensor.)
    def pool_mul(out_ap, a, b):
        nc.gpsimd.tensor_mul(out_ap, a, b)

    def pool_add(out_ap, a, b):
        nc.gpsimd.tensor_add(out_ap, a, b)

    with tile.TileContext(nc) as tc, ExitStack() as ctx:
        io_pool = ctx.enter_context(tc.tile_pool(name="io", bufs=io_bufs))
        w_pool = ctx.enter_context(tc.tile_pool(name="wts", bufs=w_bufs))
        s_pool = ctx.enter_context(tc.tile_pool(name="stk", bufs=stk_bufs))
        o_pool = ctx.enter_context(tc.tile_pool(name="out", bufs=2))

        pending = [None]

        def emit_vert(row0, r, acc_a, pacc, astk):
            # vertical contraction for an earlier chunk (deferred so the
            # accumulator-merge DMA latency hides behind the next chunk's
            # tap passes when dma_merge is on)
            nc.vector.tensor_mul(acc_a[:], astk[:, :SPL], acc_a[:])
            pool_mul(pacc[:], astk[:, SPL:], pacc[:])
            out_t = o_pool.tile([NPART, r, CPB], F16, tag="out")
            nc.vector.tensor_add(acc_a[:, :2], acc_a[:, :2], acc_a[:, 2:4])
            if out2_pool:
                pool_add(out_t[:], acc_a[:, 0], acc_a[:, 1])
            else:
                nc.vector.tensor_add(out_t[:], acc_a[:, 0], acc_a[:, 1])
            if out3_pool:
                pool_add(out_t[:], out_t[:], pacc[:, 0])
            else:
                nc.vector.tensor_add(out_t[:], out_t[:], pacc[:, 0])
            nc.sync.dma_start(_flat_src_ap(out, row0, r, W), out_t[:])

        for row0, r in chunks:
            band = io_pool.tile([NPART, r + 2 * HALO, CPB_H], F16, tag="band")
            nc.sync.dma_start(band[:], _band_src_ap(img, row0, r))
            fh_t = io_pool.tile([NPART, r, CPB], F32, tag="fh")
            nc.sync.dma_start(fh_t[:], _flat_src_ap(fh, row0, r, W))
            fw_t = io_pool.tile([NPART, r, CPB], F32, tag="fw")
            nc.sync.dma_start(fw_t[:], _flat_src_ap(fw, row0, r, W))

            if tele:
                # telescoped horizontal: difference band D[j,c] = B[j,c+1] -
                # B[j,c] (precomputed on the host, DMA'd like the band) plus
                # clamp weights c(dx) = clamp(fw - dx, 0, 1):
                #   H = B[x-2] + sum_{dx=-2..1} D[x+dx] * c(dx)
                # (exact piecewise-linear interpolation for fw in [-2, 2];
                # one fewer tap pass per engine than the tri-weight form)
                dband = io_pool.tile([NPART, r + 2 * HALO, CPB_H - 1], F16,
                                     tag="dband")
                off = row0 * (PADW - 1)
                nc.sync.dma_start(dband[:], bass_rust.AP(
                    tensor=dimg.ap().tensor, offset=off,
                    ap=[[CPB, NPART], [PADW - 1, r + 2 * HALO],
                        [1, CPB_H - 1]]))

                bstk = w_pool.tile([NPART, NTAP - 1, r, CPB], F16, tag="bstk")
                for i, dx in enumerate(range(-HALO, HALO)):
                    nc.scalar.activation(bstk[:, i], fw_t[:], RELU,
                                         bias=float(-dx), scale=1.0)
                # clamp upper bound; tensor_scalar runs in DVE 4x mode
                nc.vector.tensor_scalar_min(bstk[:], bstk[:], 1.0)
            else:
                # horizontal tri-weight stack relu(1 - |fw - dx|), dx=-2..2
                bstk = w_pool.tile([NPART, NTAP, r, CPB], F16, tag="bstk")
                for i, dx in enumerate(range(-HALO, HALO + 1)):
                    nc.scalar.activation(bstk[:, i], fw_t[:], ABS,
                                         bias=float(-dx), scale=1.0)
                nc.scalar.activation(bstk[:], bstk[:], RELU,
                                     bias=1.0, scale=-1.0)

            # vertical tri-weight stack relu(1 - |fh - dy|), dy=-2..2
            astk = w_pool.tile([NPART, NTAP, r, CPB], F16, tag="astk")
            for i, dy in enumerate(range(-HALO, HALO + 1)):
                nc.scalar.activation(astk[:, i], fh_t[:], ABS,
                                     bias=float(-dy), scale=1.0)
            nc.scalar.activation(astk[:], astk[:], RELU, bias=1.0, scale=-1.0)

            acc_a = s_pool.tile([NPART, SPL, r, CPB], F16, tag="acc_a")
            tmp = s_pool.tile([NPART, SPL, r, CPB], F16, tag="tmp")
            pacc = s_pool.tile([NPART, 1, r, CPB], F16, tag="pacc")
            ptmp = s_pool.tile([NPART, 1, r, CPB], F16, tag="ptmp")

            if tele:
                ntx = NTAP - 1
                bviews = [_stack_view(dband, CPB_H - 1, dx + HALO, NTAP, r)
                          for dx in range(-HALO, HALO)]
                basev = _stack_view(band, CPB_H, 0, NTAP, r)
            else:
                ntx = NTAP
                bviews = [_stack_view(band, CPB_H, dx + HALO, NTAP, r)
                          for dx in range(-HALO, HALO + 1)]
                basev = None
            cviews = [_bcast_planes(bstk[:, i], NTAP) for i in range(ntx)]

            # DVE: dy planes 0:4
            nc.vector.tensor_mul(acc_a[:], _sub(cviews[0], 0, SPL),
                                 _sub(bviews[0], 0, SPL))
            for i in range(1, ntx):
                nc.vector.tensor_mul(tmp[:], _sub(cviews[i], 0, SPL),
                                     _sub(bviews[i], 0, SPL))
                nc.vector.tensor_add(acc_a[:], acc_a[:], tmp[:])
                if i == 1 and pending[0] is not None:
                    emit_vert(*pending[0])
                    pending[0] = None
            if tele:
                nc.vector.tensor_add(acc_a[:], acc_a[:], _sub(basev, 0, SPL))

            # Pool: dy plane 4
            pool_mul(pacc[:], _sub(cviews[0], SPL, NTAP),
                     _sub(bviews[0], SPL, NTAP))
            for i in range(1, ntx):
                pool_mul(ptmp[:], _sub(cviews[i], SPL, NTAP),
                         _sub(bviews[i], SPL, NTAP))
                pool_add(pacc[:], pacc[:], ptmp[:])
            if tele:
                pool_add(pacc[:], pacc[:], _sub(basev, SPL, NTAP))

            pending[0] = (row0, r, acc_a, pacc, astk)

        emit_vert(*pending[0])

    nc.compile()
    return nc


def shard_inputs(input1, input2, sh=SH):
    img = np.asarray(input1, dtype=np.float32).reshape(H, W)
    flow = np.asarray(input2, dtype=np.float32).reshape(2, H, W)
    ncores = H // sh

    img_pad = np.zeros((H + 2 * HALO, PADW), dtype=np.float16)
    img_pad[HALO:H + HALO, HALO:W + HALO] = img

    # horizontal difference band of the padded image (fp16), incl. the
    # pad-boundary columns where one side is zero
    dimg_pad = np.zeros((H + 2 * HALO, PADW - 1), dtype=np.float16)
    dimg_pad[HALO:H + HALO, HALO:W + HALO - 1] = img[:, 1:] - img[:, :-1]
    dimg_pad[HALO:H + HALO, HALO - 1] = img[:, 0]
    dimg_pad[HALO:H + HALO, W + HALO - 1] = -img[:, W - 1]

    in_maps = []
    for k in range(ncores):
        h0 = k * sh
        in_maps.append({
            "img": np.ascontiguousarray(img_pad[h0:h0 + sh + 2 * HALO]),
            "dimg": np.ascontiguousarray(dimg_pad[h0:h0 + sh + 2 * HALO]),
            "fh": np.ascontiguousarray(flow[0, h0:h0 + sh]),
            "fw": np.ascontiguousarray(flow[1, h0:h0 + sh]),
        })
    return in_maps


_NC_CACHE = {}


def _patch_outliers(out, input1, input2):
    """Exact clipped-border bilinear for pixels whose flow leaves the device
    tap window.  Mirrors reference.py's math bit-for-bit in fp32."""
    f32 = np.float32
    flow = np.asarray(input2, dtype=f32).reshape(2, H, W)
    mask = (np.abs(flow[0]) > HALO) | (np.abs(flow[1]) > HALO)
    if not mask.any():
        return out
    img = np.asarray(input1, dtype=f32).reshape(H, W)
    pad = np.zeros((H + 2, W + 2), dtype=f32)
    pad[1:-1, 1:-1] = img
    hy, wx = np.nonzero(mask)
    Hu = (flow[0, hy, wx] + hy.astype(f32)).astype(f32) + f32(1.0)
    Wu = (flow[1, hy, wx] + wx.astype(f32)).astype(f32) + f32(1.0)
    hf = np.floor(Hu).astype(np.int32)
    hc = hf + 1
    wf = np.floor(Wu).astype(np.int32)
    wc = wf + 1
    hfc, hcc = np.clip(hf, 0, H + 1), np.clip(hc, 0, H + 1)
    wfc, wcc = np.clip(wf, 0, W + 1), np.clip(wc, 0, W + 1)
    dH = (hcc.astype(f32) - Hu).astype(f32)
    dW = (wcc.astype(f32) - Wu).astype(f32)
    out[hy, wx] = (
        pad[hfc, wfc] * (dW * dH)
        + pad[hcc, wfc] * (dW * (f32(1.0) - dH))
        + pad[hfc, wcc] * ((f32(1.0) - dW) * dH)
        + pad[hcc, wcc] * ((f32(1.0) - dW) * (f32(1.0) - dH))
    )
    return out


def kernel(input1, input2):
    from concourse.bass_utils import run_bass_kernel_spmd

    in_maps = shard_inputs(input1, input2)
    key = (SH, R_CHUNK, HALO)
    if key not in _NC_CACHE:
        _NC_CACHE[key] = build_nc(sh=SH, r_chunk=R_CHUNK)
    nc = _NC_CACHE[key]

    last_err = None
    for attempt in range(3):
        try:
            res = run_bass_kernel_spmd(nc, in_maps, core_ids=list(range(NCORES)))
            break
        except Exception as e:  # transient device desync — retry
            last_err = e
            time.sleep(5.0 * (attempt + 1))
    else:
        raise last_err
    out = np.concatenate([r["out"] for r in res.results], axis=0).astype(np.float32)

    out = _patch_outliers(out, input1, input2)
    return out.reshape(1, 1, H, W)


# revision 51
# speedup vs baseline: 1.0840x; 1.0132x over previous
"""Dense bilinear spatial-transformer warp — gatherless 5-row tap window on
device + host patch for outlier flow.

Device math, exact whenever both flow components lie in [-2, 2] (for N(0,1)
flow that covers ~91% of pixels; the |flow|>2 outliers are patched on the
host with the exact clipped-border gather):

  horizontal, per dy row (telescoped interpolation — 4 difference taps+base):
      H[dy] = B[y+dy, x-2] + sum_{dx=-2..1} D[y+dy, x+dx] * clamp(fw-dx, 0, 1)
      with the difference band D[j, c] = B[j, c+1] - B[j, c] precomputed on
      the host and shipped as a second fp16 input plane
  vertical (tri-weight, taps at the window edge get weight exactly 0):
      out   = sum_{dy=-2..2} relu(1 - |fh - dy|) * H[dy]

Engine split per row-chunk: the Scalar engine builds the per-axis tri-weight
stacks (Abs + Relu activations), the DVE runs the fp16 (2x-mode) tap
multiply/accumulate passes for dy planes 0:4 plus the vertical contraction,
and the GPSIMD/Pool engine independently evaluates the dy=+2 plane, which
balances the two engines' rates.  All three engines plus the DMAs pipeline
across row-chunks (triple-buffered accumulators); the first chunks are
staggered small to shorten pipeline fill.  The image ships as one fp16
zero-padded plane (halo 2); flow ships as raw fp32 row-shards (the
reference's +1 mesh shift folds into the activation biases); the output
returns as fp16 and is upcast on the host.
"""

import time
from contextlib import ExitStack

import numpy as np

import bass_rust
import concourse.bacc as bacc
import concourse.mybir as mybir
import concourse.tile as tile

F32 = mybir.dt.float32
F16 = mybir.dt.float16

H = 4096
W = 4096
NCORES = 8
SH = H // NCORES          # 512 rows per core
HALO = 2                  # tap window [-HALO, HALO] per axis
NTAP = 2 * HALO + 1       # 5
PADW = W + 2 * HALO       # padded image width (4100)
NPART = 128
CPB = W // NPART          # 32 columns per partition
CPB_H = CPB + 2 * HALO    # 36 columns incl. halo
R_CHUNK = 48
SPL = NTAP - 1            # dy planes handled by the DVE (Pool gets the last)


def _band_src_ap(t, row0, r):
    off = row0 * PADW
    return bass_rust.AP(
        tensor=t.ap().tensor, offset=off,
        ap=[[CPB, NPART], [PADW, r + 2 * HALO], [1, CPB_H]],
    )


def _flat_src_ap(t, row0, r, sh_w):
    off = row0 * sh_w
    return bass_rust.AP(
        tensor=t.ap().tensor, offset=off,
        ap=[[CPB, NPART], [sh_w, r], [1, CPB]],
    )


def _stack_view(tile_, width, col_off, nplanes, r):
    """[128, nplanes(dy), r, CPB] view; dy plane j reads rows shifted by j,
    cols shifted by col_off, of a [128, rows, width] tile."""
    base = tile_[:]
    return bass_rust.AP(
        tensor=base.tensor,
        offset=base.offset + col_off,
        ap=[list(base.ap[0]), [width, nplanes], [width, r], [1, CPB]],
    )


def _bcast_planes(ap2d, nplanes):
    return bass_rust.AP(
        tensor=ap2d.tensor, offset=ap2d.offset,
        ap=[list(ap2d.ap[0]), [0, nplanes]] + [list(d) for d in ap2d.ap[1:]],
    )


def _sub(ap, lo, hi):
    """Slice the plane dimension (axis 1) of a 4d AP."""
    return bass_rust.AP(
        tensor=ap.tensor,
        offset=ap.offset + lo * ap.ap[1][0],
        ap=[list(ap.ap[0]), [ap.ap[1][0], hi - lo]]
        + [list(d) for d in ap.ap[2:]],
    )


def build_nc(sh=SH, r_chunk=R_CHUNK, debug=False, head=(12, 20), tail=(),
             out2_pool=False, out3_pool=True, stk_bufs=2, dma_merge=False,
             io_bufs=2, w_bufs=2, tele=True, vert_at=1):
    nc = bacc.Bacc("TRN2", target_bir_lowering=False, debug=debug)
    # stagger small chunks at both ends to shorten pipeline fill and drain
    head, tail = list(head), list(tail)
    body = (sh - sum(head) - sum(tail)) // r_chunk
    assert sum(head) + sum(tail) + body * r_chunk == sh
    chunks = []
    row0 = 0
    for r in head + [r_chunk] * body + tail:
        chunks.append((row0, r))
        row0 += r

    for v in range(-HALO - 1, HALO + 2):
        val = float(v)
        if (F32, val) not in nc.const_aps.aps:
            t = nc.alloc_sbuf_tensor(f"const-float32-{val}", [128, 1], F32)
            nc.gpsimd.memset(t.ap(), val)
            nc.const_aps.aps[(F32, val)] = t.ap()
    nc.all_engine_barrier()

    img = nc.dram_tensor("img", [sh + 2 * HALO, PADW], F16, kind="ExternalInput")
    dimg = nc.dram_tensor("dimg", [sh + 2 * HALO, PADW - 1], F16,
                          kind="ExternalInput")
    fh = nc.dram_tensor("fh", [sh, W], F32, kind="ExternalInput")
    fw = nc.dram_tensor("fw", [sh, W], F32, kind="ExternalInput")
    out = nc.dram_tensor("out", [sh, W], F16, kind="ExternalOutput")

    ABS = mybir.ActivationFunctionType.Abs
    RELU = mybir.ActivationFunctionType.Relu
    MULT = mybir.AluOpType.mult
    ADD = mybir.AluOpType.add

    # (TensorScalarPtr is not a legal Pool-engine opcode on TRN2 silicon,
    # so the GPSIMD side sticks to plain tensor_tensor.)
    def pool_mul(out_ap, a, b):
        nc.gpsimd.tensor_mul(out_ap, a, b)

    def pool_add(out_ap, a, b):
        nc.gpsimd.tensor_add(out_ap, a, b)

    with tile.TileContext(nc) as tc, ExitStack() as ctx:
        io_pool = ctx.enter_context(tc.tile_pool(name="io", bufs=io_bufs))
        w_pool = ctx.enter_context(tc.tile_pool(name="wts", bufs=w_bufs))
        s_pool = ctx.enter_context(tc.tile_pool(name="stk", bufs=stk_bufs))
        o_pool = ctx.enter_context(tc.tile_pool(name="out", bufs=2))

        pending = [None]

        def emit_vert(row0, r, acc_a, pacc, astk):
            # vertical contraction for an earlier chunk (deferred so the
            # accumulator-merge DMA latency hides behind the next chunk's
            # tap passes when dma_merge is on)
            nc.vector.tensor_mul(acc_a[:], astk[:, :SPL], acc_a[:])
            pool_mul(pacc[:], astk[:, SPL:], pacc[:])
            out_t = o_pool.tile([NPART, r, CPB], F16, tag="out")
            nc.vector.tensor_add(acc_a[:, :2], acc_a[:, :2], acc_a[:, 2:4])
            if out2_pool:
                pool_add(out_t[:], acc_a[:, 0], acc_a[:, 1])
            else:
                nc.vector.tensor_add(out_t[:], acc_a[:, 0], acc_a[:, 1])
            if out3_pool:
                pool_add(out_t[:], out_t[:], pacc[:, 0])
            else:
                nc.vector.tensor_add(out_t[:], out_t[:], pacc[:, 0])
            nc.sync.dma_start(_flat_src_ap(out, row0, r, W), out_t[:])

        for row0, r in chunks:
            band = io_pool.tile([NPART, r + 2 * HALO, CPB_H], F16, tag="band")
            nc.sync.dma_start(band[:], _band_src_ap(img, row0, r))
            fh_t = io_pool.tile([NPART, r, CPB], F32, tag="fh")
            nc.sync.dma_start(fh_t[:], _flat_src_ap(fh, row0, r, W))
            fw_t = io_pool.tile([NPART, r, CPB], F32, tag="fw")
            nc.sync.dma_start(fw_t[:], _flat_src_ap(fw, row0, r, W))

            if tele:
                # telescoped horizontal: difference band D[j,c] = B[j,c+1] -
                # B[j,c] (precomputed on the host, DMA'd like the band) plus
                # clamp weights c(dx) = clamp(fw - dx, 0, 1):
                #   H = B[x-2] + sum_{dx=-2..1} D[x+dx] * c(dx)
                # (exact piecewise-linear interpolation for fw in [-2, 2];
                # one fewer tap pass per engine than the tri-weight form)
                dband = io_pool.tile([NPART, r + 2 * HALO, CPB_H - 1], F16,
                                     tag="dband")
                off = row0 * (PADW - 1)
                nc.sync.dma_start(dband[:], bass_rust.AP(
                    tensor=dimg.ap().tensor, offset=off,
                    ap=[[CPB, NPART], [PADW - 1, r + 2 * HALO],
                        [1, CPB_H - 1]]))

                bstk = w_pool.tile([NPART, NTAP - 1, r, CPB], F16, tag="bstk")
                for i, dx in enumerate(range(-HALO, HALO)):
                    nc.scalar.activation(bstk[:, i], fw_t[:], RELU,
                                         bias=float(-dx), scale=1.0)
                # clamp upper bound; tensor_scalar runs in DVE 4x mode
                nc.vector.tensor_scalar_min(bstk[:], bstk[:], 1.0)
            else:
                # horizontal tri-weight stack relu(1 - |fw - dx|), dx=-2..2
                bstk = w_pool.tile([NPART, NTAP, r, CPB], F16, tag="bstk")
                for i, dx in enumerate(range(-HALO, HALO + 1)):
                    nc.scalar.activation(bstk[:, i], fw_t[:], ABS,
                                         bias=float(-dx), scale=1.0)
                nc.scalar.activation(bstk[:], bstk[:], RELU,
                                     bias=1.0, scale=-1.0)

            # vertical tri-weight stack relu(1 - |fh - dy|), dy=-2..2
            astk = w_pool.tile([NPART, NTAP, r, CPB], F16, tag="astk")
            for i, dy in enumerate(range(-HALO, HALO + 1)):
                nc.scalar.activation(astk[:, i], fh_t[:], ABS,
                                     bias=float(-dy), scale=1.0)
            nc.scalar.activation(astk[:], astk[:], RELU, bias=1.0, scale=-1.0)

            acc_a = s_pool.tile([NPART, SPL, r, CPB], F16, tag="acc_a")
            tmp = s_pool.tile([NPART, SPL, r, CPB], F16, tag="tmp")
            pacc = s_pool.tile([NPART, 1, r, CPB], F16, tag="pacc")
            ptmp = s_pool.tile([NPART, 1, r, CPB], F16, tag="ptmp")

            if tele:
                ntx = NTAP - 1
                bviews = [_stack_view(dband, CPB_H - 1, dx + HALO, NTAP, r)
                          for dx in range(-HALO, HALO)]
                basev = _stack_view(band, CPB_H, 0, NTAP, r)
            else:
                ntx = NTAP
                bviews = [_stack_view(band, CPB_H, dx + HALO, NTAP, r)
                          for dx in range(-HALO, HALO + 1)]
                basev = None
            cviews = [_bcast_planes(bstk[:, i], NTAP) for i in range(ntx)]

            # DVE: dy planes 0:4
            nc.vector.tensor_mul(acc_a[:], _sub(cviews[0], 0, SPL),
                                 _sub(bviews[0], 0, SPL))
            for i in range(1, ntx):
                nc.vector.tensor_mul(tmp[:], _sub(cviews[i], 0, SPL),
                                     _sub(bviews[i], 0, SPL))
                nc.vector.tensor_add(acc_a[:], acc_a[:], tmp[:])
                if i == vert_at and pending[0] is not None:
                    emit_vert(*pending[0])
                    pending[0] = None
            if tele:
                nc.vector.tensor_add(acc_a[:], acc_a[:], _sub(basev, 0, SPL))

            # Pool: dy plane 4
            pool_mul(pacc[:], _sub(cviews[0], SPL, NTAP),
                     _sub(bviews[0], SPL, NTAP))
            for i in range(1, ntx):
                pool_mul(ptmp[:], _sub(cviews[i], SPL, NTAP),
                         _sub(bviews[i], SPL, NTAP))
                pool_add(pacc[:], pacc[:], ptmp[:])
            if tele:
                pool_add(pacc[:], pacc[:], _sub(basev, SPL, NTAP))

            pending[0] = (row0, r, acc_a, pacc, astk)

        emit_vert(*pending[0])

    nc.compile()
    return nc


def shard_inputs(input1, input2, sh=SH):
    img = np.asarray(input1, dtype=np.float32).reshape(H, W)
    flow = np.asarray(input2, dtype=np.float32).reshape(2, H, W)
    ncores = H // sh

    img_pad = np.zeros((H + 2 * HALO, PADW), dtype=np.float16)
    img_pad[HALO:H + HALO, HALO:W + HALO] = img

    # horizontal difference band of the padded image (fp16), incl. the
    # pad-boundary columns where one side is zero
    dimg_pad = np.zeros((H + 2 * HALO, PADW - 1), dtype=np.float16)
    dimg_pad[HALO:H + HALO, HALO:W + HALO - 1] = img[:, 1:] - img[:, :-1]
    dimg_pad[HALO:H + HALO, HALO - 1] = img[:, 0]
    dimg_pad[HALO:H + HALO, W + HALO - 1] = -img[:, W - 1]

    in_maps = []
    for k in range(ncores):
        h0 = k * sh
        in_maps.append({
            "img": np.ascontiguousarray(img_pad[h0:h0 + sh + 2 * HALO]),
            "dimg": np.ascontiguousarray(dimg_pad[h0:h0 + sh + 2 * HALO]),
            "fh": np.ascontiguousarray(flow[0, h0:h0 + sh]),
            "fw": np.ascontiguousarray(flow[1, h0:h0 + sh]),
        })
    return in_maps


_NC_CACHE = {}


def _patch_outliers(out, input1, input2):
    """Exact clipped-border bilinear for pixels whose flow leaves the device
    tap window.  Mirrors reference.py's math bit-for-bit in fp32."""
    f32 = np.float32
    flow = np.asarray(input2, dtype=f32).reshape(2, H, W)
    mask = (np.abs(flow[0]) > HALO) | (np.abs(flow[1]) > HALO)
    if not mask.any():
        return out
    img = np.asarray(input1, dtype=f32).reshape(H, W)
    pad = np.zeros((H + 2, W + 2), dtype=f32)
    pad[1:-1, 1:-1] = img
    hy, wx = np.nonzero(mask)
    Hu = (flow[0, hy, wx] + hy.astype(f32)).astype(f32) + f32(1.0)
    Wu = (flow[1, hy, wx] + wx.astype(f32)).astype(f32) + f32(1.0)
    hf = np.floor(Hu).astype(np.int32)
    hc = hf + 1
    wf = np.floor(Wu).astype(np.int32)
    wc = wf + 1
    hfc, hcc = np.clip(hf, 0, H + 1), np.clip(hc, 0, H + 1)
    wfc, wcc = np.clip(wf, 0, W + 1), np.clip(wc, 0, W + 1)
    dH = (hcc.astype(f32) - Hu).astype(f32)
    dW = (wcc.astype(f32) - Wu).astype(f32)
    out[hy, wx] = (
        pad[hfc, wfc] * (dW * dH)
        + pad[hcc, wfc] * (dW * (f32(1.0) - dH))
        + pad[hfc, wcc] * ((f32(1.0) - dW) * dH)
        + pad[hcc, wcc] * ((f32(1.0) - dW) * (f32(1.0) - dH))
    )
    return out


def kernel(input1, input2):
    from concourse.bass_utils import run_bass_kernel_spmd

    in_maps = shard_inputs(input1, input2)
    key = (SH, R_CHUNK, HALO)
    if key not in _NC_CACHE:
        _NC_CACHE[key] = build_nc(sh=SH, r_chunk=R_CHUNK)
    nc = _NC_CACHE[key]

    last_err = None
    for attempt in range(3):
        try:
            res = run_bass_kernel_spmd(nc, in_maps, core_ids=list(range(NCORES)))
            break
        except Exception as e:  # transient device desync — retry
            last_err = e
            time.sleep(5.0 * (attempt + 1))
    else:
        raise last_err
    out = np.concatenate([r["out"] for r in res.results], axis=0).astype(np.float32)

    out = _patch_outliers(out, input1, input2)
    return out.reshape(1, 1, H, W)


# revision 52
# speedup vs baseline: 1.0915x; 1.0069x over previous
"""Dense bilinear spatial-transformer warp — gatherless 5-row tap window on
device + host patch for outlier flow.

Device math, exact whenever both flow components lie in [-2, 2] (for N(0,1)
flow that covers ~91% of pixels; the |flow|>2 outliers are patched on the
host with the exact clipped-border gather):

  horizontal, per dy row (telescoped interpolation — 4 difference taps+base):
      H[dy] = B[y+dy, x-2] + sum_{dx=-2..1} D[y+dy, x+dx] * clamp(fw-dx, 0, 1)
      with the difference band D[j, c] = B[j, c+1] - B[j, c] precomputed on
      the host and shipped as a second fp16 input plane
  vertical (tri-weight, taps at the window edge get weight exactly 0):
      out   = sum_{dy=-2..2} relu(1 - |fh - dy|) * H[dy]

Engine split per row-chunk: the Scalar engine builds the per-axis tri-weight
stacks (Abs + Relu activations), the DVE runs the fp16 (2x-mode) tap
multiply/accumulate passes for dy planes 0:4 plus the vertical contraction,
and the GPSIMD/Pool engine independently evaluates the dy=+2 plane, which
balances the two engines' rates.  All three engines plus the DMAs pipeline
across row-chunks (triple-buffered accumulators); the first chunks are
staggered small to shorten pipeline fill.  The image ships as one fp16
zero-padded plane (halo 2); flow ships as raw fp32 row-shards (the
reference's +1 mesh shift folds into the activation biases); the output
returns as fp16 and is upcast on the host.
"""

import time
from contextlib import ExitStack

import numpy as np

import bass_rust
import concourse.bacc as bacc
import concourse.mybir as mybir
import concourse.tile as tile

F32 = mybir.dt.float32
F16 = mybir.dt.float16

H = 4096
W = 4096
NCORES = 8
SH = H // NCORES          # 512 rows per core
HALO = 2                  # tap window [-HALO, HALO] per axis
NTAP = 2 * HALO + 1       # 5
PADW = W + 2 * HALO       # padded image width (4100)
NPART = 128
CPB = W // NPART          # 32 columns per partition
CPB_H = CPB + 2 * HALO    # 36 columns incl. halo
R_CHUNK = 48
SPL = NTAP - 1            # dy planes handled by the DVE (Pool gets the last)


def _band_src_ap(t, row0, r):
    off = row0 * PADW
    return bass_rust.AP(
        tensor=t.ap().tensor, offset=off,
        ap=[[CPB, NPART], [PADW, r + 2 * HALO], [1, CPB_H]],
    )


def _flat_src_ap(t, row0, r, sh_w):
    off = row0 * sh_w
    return bass_rust.AP(
        tensor=t.ap().tensor, offset=off,
        ap=[[CPB, NPART], [sh_w, r], [1, CPB]],
    )


def _stack_view(tile_, width, col_off, nplanes, r):
    """[128, nplanes(dy), r, CPB] view; dy plane j reads rows shifted by j,
    cols shifted by col_off, of a [128, rows, width] tile."""
    base = tile_[:]
    return bass_rust.AP(
        tensor=base.tensor,
        offset=base.offset + col_off,
        ap=[list(base.ap[0]), [width, nplanes], [width, r], [1, CPB]],
    )


def _bcast_planes(ap2d, nplanes):
    return bass_rust.AP(
        tensor=ap2d.tensor, offset=ap2d.offset,
        ap=[list(ap2d.ap[0]), [0, nplanes]] + [list(d) for d in ap2d.ap[1:]],
    )


def _sub(ap, lo, hi):
    """Slice the plane dimension (axis 1) of a 4d AP."""
    return bass_rust.AP(
        tensor=ap.tensor,
        offset=ap.offset + lo * ap.ap[1][0],
        ap=[list(ap.ap[0]), [ap.ap[1][0], hi - lo]]
        + [list(d) for d in ap.ap[2:]],
    )


def build_nc(sh=SH, r_chunk=R_CHUNK, debug=False, head=(12, 20), tail=(),
             out2_pool=False, out3_pool=True, stk_bufs=2, dma_merge=False,
             io_bufs=2, w_bufs=2, tele=True, vert_at=1):
    nc = bacc.Bacc("TRN2", target_bir_lowering=False, debug=debug)
    # stagger small chunks at both ends to shorten pipeline fill and drain
    head, tail = list(head), list(tail)
    body = (sh - sum(head) - sum(tail)) // r_chunk
    assert sum(head) + sum(tail) + body * r_chunk == sh
    chunks = []
    row0 = 0
    for r in head + [r_chunk] * body + tail:
        chunks.append((row0, r))
        row0 += r

    for v in range(-HALO - 1, HALO + 2):
        val = float(v)
        if (F32, val) not in nc.const_aps.aps:
            t = nc.alloc_sbuf_tensor(f"const-float32-{val}", [128, 1], F32)
            nc.gpsimd.memset(t.ap(), val)
            nc.const_aps.aps[(F32, val)] = t.ap()
    nc.all_engine_barrier()

    img = nc.dram_tensor("img", [sh + 2 * HALO, PADW], F16, kind="ExternalInput")
    dimg = nc.dram_tensor("dimg", [sh + 2 * HALO, PADW - 1], F16,
                          kind="ExternalInput")
    fh = nc.dram_tensor("fh", [sh, W], F32, kind="ExternalInput")
    fw = nc.dram_tensor("fw", [sh, W], F32, kind="ExternalInput")
    out = nc.dram_tensor("out", [sh, W], F16, kind="ExternalOutput")

    ABS = mybir.ActivationFunctionType.Abs
    RELU = mybir.ActivationFunctionType.Relu
    MULT = mybir.AluOpType.mult
    ADD = mybir.AluOpType.add

    # (TensorScalarPtr is not a legal Pool-engine opcode on TRN2 silicon,
    # so the GPSIMD side sticks to plain tensor_tensor.)
    def pool_mul(out_ap, a, b):
        nc.gpsimd.tensor_mul(out_ap, a, b)

    def pool_add(out_ap, a, b):
        nc.gpsimd.tensor_add(out_ap, a, b)

    with tile.TileContext(nc) as tc, ExitStack() as ctx:
        io_pool = ctx.enter_context(tc.tile_pool(name="io", bufs=io_bufs))
        w_pool = ctx.enter_context(tc.tile_pool(name="wts", bufs=w_bufs))
        s_pool = ctx.enter_context(tc.tile_pool(name="stk", bufs=stk_bufs))
        o_pool = ctx.enter_context(tc.tile_pool(name="out", bufs=2))

        pending = [None]

        def emit_vert(row0, r, acc_a, pacc, astk):
            # vertical contraction for an earlier chunk (deferred so the
            # accumulator-merge DMA latency hides behind the next chunk's
            # tap passes when dma_merge is on)
            nc.vector.tensor_mul(acc_a[:], astk[:, :SPL], acc_a[:])
            pool_mul(pacc[:], astk[:, SPL:], pacc[:])
            out_t = o_pool.tile([NPART, r, CPB], F16, tag="out")
            nc.vector.tensor_add(acc_a[:, :2], acc_a[:, :2], acc_a[:, 2:4])
            if out2_pool:
                pool_add(out_t[:], acc_a[:, 0], acc_a[:, 1])
            else:
                nc.vector.tensor_add(out_t[:], acc_a[:, 0], acc_a[:, 1])
            if out3_pool:
                pool_add(out_t[:], out_t[:], pacc[:, 0])
            else:
                nc.vector.tensor_add(out_t[:], out_t[:], pacc[:, 0])
            nc.sync.dma_start(_flat_src_ap(out, row0, r, W), out_t[:])

        for row0, r in chunks:
            band = io_pool.tile([NPART, r + 2 * HALO, CPB_H], F16, tag="band")
            nc.sync.dma_start(band[:], _band_src_ap(img, row0, r))
            fh_t = io_pool.tile([NPART, r, CPB], F32, tag="fh")
            nc.sync.dma_start(fh_t[:], _flat_src_ap(fh, row0, r, W))
            fw_t = io_pool.tile([NPART, r, CPB], F32, tag="fw")
            nc.sync.dma_start(fw_t[:], _flat_src_ap(fw, row0, r, W))

            if tele:
                # telescoped horizontal: difference band D[j,c] = B[j,c+1] -
                # B[j,c] (precomputed on the host, DMA'd like the band) plus
                # clamp weights c(dx) = clamp(fw - dx, 0, 1):
                #   H = B[x-2] + sum_{dx=-2..1} D[x+dx] * c(dx)
                # (exact piecewise-linear interpolation for fw in [-2, 2];
                # one fewer tap pass per engine than the tri-weight form)
                dband = io_pool.tile([NPART, r + 2 * HALO, CPB_H - 1], F16,
                                     tag="dband")
                off = row0 * (PADW - 1)
                nc.sync.dma_start(dband[:], bass_rust.AP(
                    tensor=dimg.ap().tensor, offset=off,
                    ap=[[CPB, NPART], [PADW - 1, r + 2 * HALO],
                        [1, CPB_H - 1]]))

                bstk = w_pool.tile([NPART, NTAP - 1, r, CPB], F16, tag="bstk")
                for i, dx in enumerate(range(-HALO, HALO)):
                    nc.scalar.activation(bstk[:, i], fw_t[:], RELU,
                                         bias=float(-dx), scale=1.0)
                # clamp upper bound; tensor_scalar runs in DVE 4x mode
                # top tap dx=+1 needs no upper clamp: fw <= 2 in-window means
                # relu(fw-1) <= 1 already (out-of-window pixels stay finite
                # and are host-patched)
                nc.vector.tensor_scalar_min(bstk[:, :NTAP - 2],
                                            bstk[:, :NTAP - 2], 1.0)
            else:
                # horizontal tri-weight stack relu(1 - |fw - dx|), dx=-2..2
                bstk = w_pool.tile([NPART, NTAP, r, CPB], F16, tag="bstk")
                for i, dx in enumerate(range(-HALO, HALO + 1)):
                    nc.scalar.activation(bstk[:, i], fw_t[:], ABS,
                                         bias=float(-dx), scale=1.0)
                nc.scalar.activation(bstk[:], bstk[:], RELU,
                                     bias=1.0, scale=-1.0)

            # vertical tri-weight stack relu(1 - |fh - dy|), dy=-2..2
            astk = w_pool.tile([NPART, NTAP, r, CPB], F16, tag="astk")
            for i, dy in enumerate(range(-HALO, HALO + 1)):
                nc.scalar.activation(astk[:, i], fh_t[:], ABS,
                                     bias=float(-dy), scale=1.0)
            nc.scalar.activation(astk[:], astk[:], RELU, bias=1.0, scale=-1.0)

            acc_a = s_pool.tile([NPART, SPL, r, CPB], F16, tag="acc_a")
            tmp = s_pool.tile([NPART, SPL, r, CPB], F16, tag="tmp")
            pacc = s_pool.tile([NPART, 1, r, CPB], F16, tag="pacc")
            ptmp = s_pool.tile([NPART, 1, r, CPB], F16, tag="ptmp")

            if tele:
                ntx = NTAP - 1
                bviews = [_stack_view(dband, CPB_H - 1, dx + HALO, NTAP, r)
                          for dx in range(-HALO, HALO)]
                basev = _stack_view(band, CPB_H, 0, NTAP, r)
            else:
                ntx = NTAP
                bviews = [_stack_view(band, CPB_H, dx + HALO, NTAP, r)
                          for dx in range(-HALO, HALO + 1)]
                basev = None
            cviews = [_bcast_planes(bstk[:, i], NTAP) for i in range(ntx)]

            # DVE: dy planes 0:4
            nc.vector.tensor_mul(acc_a[:], _sub(cviews[0], 0, SPL),
                                 _sub(bviews[0], 0, SPL))
            for i in range(1, ntx):
                nc.vector.tensor_mul(tmp[:], _sub(cviews[i], 0, SPL),
                                     _sub(bviews[i], 0, SPL))
                nc.vector.tensor_add(acc_a[:], acc_a[:], tmp[:])
                if i == vert_at and pending[0] is not None:
                    emit_vert(*pending[0])
                    pending[0] = None
            if tele:
                nc.vector.tensor_add(acc_a[:], acc_a[:], _sub(basev, 0, SPL))

            # Pool: dy plane 4
            pool_mul(pacc[:], _sub(cviews[0], SPL, NTAP),
                     _sub(bviews[0], SPL, NTAP))
            for i in range(1, ntx):
                pool_mul(ptmp[:], _sub(cviews[i], SPL, NTAP),
                         _sub(bviews[i], SPL, NTAP))
                pool_add(pacc[:], pacc[:], ptmp[:])
            if tele:
                pool_add(pacc[:], pacc[:], _sub(basev, SPL, NTAP))

            pending[0] = (row0, r, acc_a, pacc, astk)

        emit_vert(*pending[0])

    nc.compile()
    return nc


def shard_inputs(input1, input2, sh=SH):
    img = np.asarray(input1, dtype=np.float32).reshape(H, W)
    flow = np.asarray(input2, dtype=np.float32).reshape(2, H, W)
    ncores = H // sh

    img_pad = np.zeros((H + 2 * HALO, PADW), dtype=np.float16)
    img_pad[HALO:H + HALO, HALO:W + HALO] = img

    # horizontal difference band of the padded image (fp16), incl. the
    # pad-boundary columns where one side is zero
    dimg_pad = np.zeros((H + 2 * HALO, PADW - 1), dtype=np.float16)
    dimg_pad[HALO:H + HALO, HALO:W + HALO - 1] = img[:, 1:] - img[:, :-1]
    dimg_pad[HALO:H + HALO, HALO - 1] = img[:, 0]
    dimg_pad[HALO:H + HALO, W + HALO - 1] = -img[:, W - 1]

    in_maps = []
    for k in range(ncores):
        h0 = k * sh
        in_maps.append({
            "img": np.ascontiguousarray(img_pad[h0:h0 + sh + 2 * HALO]),
            "dimg": np.ascontiguousarray(dimg_pad[h0:h0 + sh + 2 * HALO]),
            "fh": np.ascontiguousarray(flow[0, h0:h0 + sh]),
            "fw": np.ascontiguousarray(flow[1, h0:h0 + sh]),
        })
    return in_maps


_NC_CACHE = {}


def _patch_outliers(out, input1, input2):
    """Exact clipped-border bilinear for pixels whose flow leaves the device
    tap window.  Mirrors reference.py's math bit-for-bit in fp32."""
    f32 = np.float32
    flow = np.asarray(input2, dtype=f32).reshape(2, H, W)
    mask = (np.abs(flow[0]) > HALO) | (np.abs(flow[1]) > HALO)
    if not mask.any():
        return out
    img = np.asarray(input1, dtype=f32).reshape(H, W)
    pad = np.zeros((H + 2, W + 2), dtype=f32)
    pad[1:-1, 1:-1] = img
    hy, wx = np.nonzero(mask)
    Hu = (flow[0, hy, wx] + hy.astype(f32)).astype(f32) + f32(1.0)
    Wu = (flow[1, hy, wx] + wx.astype(f32)).astype(f32) + f32(1.0)
    hf = np.floor(Hu).astype(np.int32)
    hc = hf + 1
    wf = np.floor(Wu).astype(np.int32)
    wc = wf + 1
    hfc, hcc = np.clip(hf, 0, H + 1), np.clip(hc, 0, H + 1)
    wfc, wcc = np.clip(wf, 0, W + 1), np.clip(wc, 0, W + 1)
    dH = (hcc.astype(f32) - Hu).astype(f32)
    dW = (wcc.astype(f32) - Wu).astype(f32)
    out[hy, wx] = (
        pad[hfc, wfc] * (dW * dH)
        + pad[hcc, wfc] * (dW * (f32(1.0) - dH))
        + pad[hfc, wcc] * ((f32(1.0) - dW) * dH)
        + pad[hcc, wcc] * ((f32(1.0) - dW) * (f32(1.0) - dH))
    )
    return out


def kernel(input1, input2):
    from concourse.bass_utils import run_bass_kernel_spmd

    in_maps = shard_inputs(input1, input2)
    key = (SH, R_CHUNK, HALO)
    if key not in _NC_CACHE:
        _NC_CACHE[key] = build_nc(sh=SH, r_chunk=R_CHUNK)
    nc = _NC_CACHE[key]

    last_err = None
    for attempt in range(3):
        try:
            res = run_bass_kernel_spmd(nc, in_maps, core_ids=list(range(NCORES)))
            break
        except Exception as e:  # transient device desync — retry
            last_err = e
            time.sleep(5.0 * (attempt + 1))
    else:
        raise last_err
    out = np.concatenate([r["out"] for r in res.results], axis=0).astype(np.float32)

    out = _patch_outliers(out, input1, input2)
    return out.reshape(1, 1, H, W)


# revision 53
# speedup vs baseline: 1.0954x; 1.0036x over previous
"""Dense bilinear spatial-transformer warp — gatherless 5-row tap window on
device + host patch for outlier flow.

Device math, exact whenever both flow components lie in [-2, 2] (for N(0,1)
flow that covers ~91% of pixels; the |flow|>2 outliers are patched on the
host with the exact clipped-border gather):

  horizontal, per dy row (telescoped interpolation — 4 difference taps+base):
      H[dy] = B[y+dy, x-2] + sum_{dx=-2..1} D[y+dy, x+dx] * clamp(fw-dx, 0, 1)
      with the difference band D[j, c] = B[j, c+1] - B[j, c] precomputed on
      the host and shipped as a second fp16 input plane
  vertical (tri-weight, taps at the window edge get weight exactly 0):
      out   = sum_{dy=-2..2} relu(1 - |fh - dy|) * H[dy]

Engine split per row-chunk: the Scalar engine builds the per-axis tri-weight
stacks (Abs + Relu activations), the DVE runs the fp16 (2x-mode) tap
multiply/accumulate passes for dy planes 0:4 plus the vertical contraction,
and the GPSIMD/Pool engine independently evaluates the dy=+2 plane, which
balances the two engines' rates.  All three engines plus the DMAs pipeline
across row-chunks (triple-buffered accumulators); the first chunks are
staggered small to shorten pipeline fill.  The image ships as one fp16
zero-padded plane (halo 2); flow ships as raw fp32 row-shards (the
reference's +1 mesh shift folds into the activation biases); the output
returns as fp16 and is upcast on the host.
"""

import time
from contextlib import ExitStack

import numpy as np

import bass_rust
import concourse.bacc as bacc
import concourse.mybir as mybir
import concourse.tile as tile

F32 = mybir.dt.float32
F16 = mybir.dt.float16

H = 4096
W = 4096
NCORES = 8
SH = H // NCORES          # 512 rows per core
HALO = 2                  # tap window [-HALO, HALO] per axis
NTAP = 2 * HALO + 1       # 5
PADW = W + 2 * HALO       # padded image width (4100)
NPART = 128
CPB = W // NPART          # 32 columns per partition
CPB_H = CPB + 2 * HALO    # 36 columns incl. halo
R_CHUNK = 48
SPL = NTAP - 1            # dy planes handled by the DVE (Pool gets the last)


def _band_src_ap(t, row0, r):
    off = row0 * PADW
    return bass_rust.AP(
        tensor=t.ap().tensor, offset=off,
        ap=[[CPB, NPART], [PADW, r + 2 * HALO], [1, CPB_H]],
    )


def _flat_src_ap(t, row0, r, sh_w):
    off = row0 * sh_w
    return bass_rust.AP(
        tensor=t.ap().tensor, offset=off,
        ap=[[CPB, NPART], [sh_w, r], [1, CPB]],
    )


def _stack_view(tile_, width, col_off, nplanes, r):
    """[128, nplanes(dy), r, CPB] view; dy plane j reads rows shifted by j,
    cols shifted by col_off, of a [128, rows, width] tile."""
    base = tile_[:]
    return bass_rust.AP(
        tensor=base.tensor,
        offset=base.offset + col_off,
        ap=[list(base.ap[0]), [width, nplanes], [width, r], [1, CPB]],
    )


def _bcast_planes(ap2d, nplanes):
    return bass_rust.AP(
        tensor=ap2d.tensor, offset=ap2d.offset,
        ap=[list(ap2d.ap[0]), [0, nplanes]] + [list(d) for d in ap2d.ap[1:]],
    )


def _sub(ap, lo, hi):
    """Slice the plane dimension (axis 1) of a 4d AP."""
    return bass_rust.AP(
        tensor=ap.tensor,
        offset=ap.offset + lo * ap.ap[1][0],
        ap=[list(ap.ap[0]), [ap.ap[1][0], hi - lo]]
        + [list(d) for d in ap.ap[2:]],
    )


def build_nc(sh=SH, r_chunk=R_CHUNK, debug=False, head=(14, 18), tail=(),
             out2_pool=False, out3_pool=True, stk_bufs=2, dma_merge=False,
             io_bufs=2, w_bufs=2, tele=True, vert_at=1):
    nc = bacc.Bacc("TRN2", target_bir_lowering=False, debug=debug)
    # stagger small chunks at both ends to shorten pipeline fill and drain
    head, tail = list(head), list(tail)
    body = (sh - sum(head) - sum(tail)) // r_chunk
    assert sum(head) + sum(tail) + body * r_chunk == sh
    chunks = []
    row0 = 0
    for r in head + [r_chunk] * body + tail:
        chunks.append((row0, r))
        row0 += r

    for v in range(-HALO - 1, HALO + 2):
        val = float(v)
        if (F32, val) not in nc.const_aps.aps:
            t = nc.alloc_sbuf_tensor(f"const-float32-{val}", [128, 1], F32)
            nc.gpsimd.memset(t.ap(), val)
            nc.const_aps.aps[(F32, val)] = t.ap()
    nc.all_engine_barrier()

    img = nc.dram_tensor("img", [sh + 2 * HALO, PADW], F16, kind="ExternalInput")
    dimg = nc.dram_tensor("dimg", [sh + 2 * HALO, PADW - 1], F16,
                          kind="ExternalInput")
    fh = nc.dram_tensor("fh", [sh, W], F32, kind="ExternalInput")
    fw = nc.dram_tensor("fw", [sh, W], F32, kind="ExternalInput")
    out = nc.dram_tensor("out", [sh, W], F16, kind="ExternalOutput")

    ABS = mybir.ActivationFunctionType.Abs
    RELU = mybir.ActivationFunctionType.Relu
    MULT = mybir.AluOpType.mult
    ADD = mybir.AluOpType.add

    # (TensorScalarPtr is not a legal Pool-engine opcode on TRN2 silicon,
    # so the GPSIMD side sticks to plain tensor_tensor.)
    def pool_mul(out_ap, a, b):
        nc.gpsimd.tensor_mul(out_ap, a, b)

    def pool_add(out_ap, a, b):
        nc.gpsimd.tensor_add(out_ap, a, b)

    with tile.TileContext(nc) as tc, ExitStack() as ctx:
        io_pool = ctx.enter_context(tc.tile_pool(name="io", bufs=io_bufs))
        w_pool = ctx.enter_context(tc.tile_pool(name="wts", bufs=w_bufs))
        s_pool = ctx.enter_context(tc.tile_pool(name="stk", bufs=stk_bufs))
        o_pool = ctx.enter_context(tc.tile_pool(name="out", bufs=2))

        pending = [None]

        def emit_vert(row0, r, acc_a, pacc, astk):
            # vertical contraction for an earlier chunk (deferred so the
            # accumulator-merge DMA latency hides behind the next chunk's
            # tap passes when dma_merge is on)
            nc.vector.tensor_mul(acc_a[:], astk[:, :SPL], acc_a[:])
            pool_mul(pacc[:], astk[:, SPL:], pacc[:])
            out_t = o_pool.tile([NPART, r, CPB], F16, tag="out")
            nc.vector.tensor_add(acc_a[:, :2], acc_a[:, :2], acc_a[:, 2:4])
            if out2_pool:
                pool_add(out_t[:], acc_a[:, 0], acc_a[:, 1])
            else:
                nc.vector.tensor_add(out_t[:], acc_a[:, 0], acc_a[:, 1])
            if out3_pool:
                pool_add(out_t[:], out_t[:], pacc[:, 0])
            else:
                nc.vector.tensor_add(out_t[:], out_t[:], pacc[:, 0])
            nc.sync.dma_start(_flat_src_ap(out, row0, r, W), out_t[:])

        for row0, r in chunks:
            band = io_pool.tile([NPART, r + 2 * HALO, CPB_H], F16, tag="band")
            nc.sync.dma_start(band[:], _band_src_ap(img, row0, r))
            fh_t = io_pool.tile([NPART, r, CPB], F32, tag="fh")
            nc.sync.dma_start(fh_t[:], _flat_src_ap(fh, row0, r, W))
            fw_t = io_pool.tile([NPART, r, CPB], F32, tag="fw")
            nc.sync.dma_start(fw_t[:], _flat_src_ap(fw, row0, r, W))

            if tele:
                # telescoped horizontal: difference band D[j,c] = B[j,c+1] -
                # B[j,c] (precomputed on the host, DMA'd like the band) plus
                # clamp weights c(dx) = clamp(fw - dx, 0, 1):
                #   H = B[x-2] + sum_{dx=-2..1} D[x+dx] * c(dx)
                # (exact piecewise-linear interpolation for fw in [-2, 2];
                # one fewer tap pass per engine than the tri-weight form)
                dband = io_pool.tile([NPART, r + 2 * HALO, CPB_H - 1], F16,
                                     tag="dband")
                off = row0 * (PADW - 1)
                nc.sync.dma_start(dband[:], bass_rust.AP(
                    tensor=dimg.ap().tensor, offset=off,
                    ap=[[CPB, NPART], [PADW - 1, r + 2 * HALO],
                        [1, CPB_H - 1]]))

                bstk = w_pool.tile([NPART, NTAP - 1, r, CPB], F16, tag="bstk")
                for i, dx in enumerate(range(-HALO, HALO)):
                    nc.scalar.activation(bstk[:, i], fw_t[:], RELU,
                                         bias=float(-dx), scale=1.0)
                # clamp upper bound; tensor_scalar runs in DVE 4x mode
                # top tap dx=+1 needs no upper clamp: fw <= 2 in-window means
                # relu(fw-1) <= 1 already (out-of-window pixels stay finite
                # and are host-patched)
                nc.vector.tensor_scalar_min(bstk[:, :NTAP - 2],
                                            bstk[:, :NTAP - 2], 1.0)
            else:
                # horizontal tri-weight stack relu(1 - |fw - dx|), dx=-2..2
                bstk = w_pool.tile([NPART, NTAP, r, CPB], F16, tag="bstk")
                for i, dx in enumerate(range(-HALO, HALO + 1)):
                    nc.scalar.activation(bstk[:, i], fw_t[:], ABS,
                                         bias=float(-dx), scale=1.0)
                nc.scalar.activation(bstk[:], bstk[:], RELU,
                                     bias=1.0, scale=-1.0)

            # vertical tri-weight stack relu(1 - |fh - dy|), dy=-2..2
            astk = w_pool.tile([NPART, NTAP, r, CPB], F16, tag="astk")
            for i, dy in enumerate(range(-HALO, HALO + 1)):
                nc.scalar.activation(astk[:, i], fh_t[:], ABS,
                                     bias=float(-dy), scale=1.0)
            nc.scalar.activation(astk[:], astk[:], RELU, bias=1.0, scale=-1.0)

            acc_a = s_pool.tile([NPART, SPL, r, CPB], F16, tag="acc_a")
            tmp = s_pool.tile([NPART, SPL, r, CPB], F16, tag="tmp")
            pacc = s_pool.tile([NPART, 1, r, CPB], F16, tag="pacc")
            ptmp = s_pool.tile([NPART, 1, r, CPB], F16, tag="ptmp")

            if tele:
                ntx = NTAP - 1
                bviews = [_stack_view(dband, CPB_H - 1, dx + HALO, NTAP, r)
                          for dx in range(-HALO, HALO)]
                basev = _stack_view(band, CPB_H, 0, NTAP, r)
            else:
                ntx = NTAP
                bviews = [_stack_view(band, CPB_H, dx + HALO, NTAP, r)
                          for dx in range(-HALO, HALO + 1)]
                basev = None
            cviews = [_bcast_planes(bstk[:, i], NTAP) for i in range(ntx)]

            # DVE: dy planes 0:4
            nc.vector.tensor_mul(acc_a[:], _sub(cviews[0], 0, SPL),
                                 _sub(bviews[0], 0, SPL))
            for i in range(1, ntx):
                nc.vector.tensor_mul(tmp[:], _sub(cviews[i], 0, SPL),
                                     _sub(bviews[i], 0, SPL))
                nc.vector.tensor_add(acc_a[:], acc_a[:], tmp[:])
                if i == vert_at and pending[0] is not None:
                    emit_vert(*pending[0])
                    pending[0] = None
            if tele:
                nc.vector.tensor_add(acc_a[:], acc_a[:], _sub(basev, 0, SPL))

            # Pool: dy plane 4
            pool_mul(pacc[:], _sub(cviews[0], SPL, NTAP),
                     _sub(bviews[0], SPL, NTAP))
            for i in range(1, ntx):
                pool_mul(ptmp[:], _sub(cviews[i], SPL, NTAP),
                         _sub(bviews[i], SPL, NTAP))
                pool_add(pacc[:], pacc[:], ptmp[:])
            if tele:
                pool_add(pacc[:], pacc[:], _sub(basev, SPL, NTAP))

            pending[0] = (row0, r, acc_a, pacc, astk)

        emit_vert(*pending[0])

    nc.compile()
    return nc


def shard_inputs(input1, input2, sh=SH):
    img = np.asarray(input1, dtype=np.float32).reshape(H, W)
    flow = np.asarray(input2, dtype=np.float32).reshape(2, H, W)
    ncores = H // sh

    img_pad = np.zeros((H + 2 * HALO, PADW), dtype=np.float16)
    img_pad[HALO:H + HALO, HALO:W + HALO] = img

    # horizontal difference band of the padded image (fp16), incl. the
    # pad-boundary columns where one side is zero
    dimg_pad = np.zeros((H + 2 * HALO, PADW - 1), dtype=np.float16)
    dimg_pad[HALO:H + HALO, HALO:W + HALO - 1] = img[:, 1:] - img[:, :-1]
    dimg_pad[HALO:H + HALO, HALO - 1] = img[:, 0]
    dimg_pad[HALO:H + HALO, W + HALO - 1] = -img[:, W - 1]

    in_maps = []
    for k in range(ncores):
        h0 = k * sh
        in_maps.append({
            "img": np.ascontiguousarray(img_pad[h0:h0 + sh + 2 * HALO]),
            "dimg": np.ascontiguousarray(dimg_pad[h0:h0 + sh + 2 * HALO]),
            "fh": np.ascontiguousarray(flow[0, h0:h0 + sh]),
            "fw": np.ascontiguousarray(flow[1, h0:h0 + sh]),
        })
    return in_maps


_NC_CACHE = {}


def _patch_outliers(out, input1, input2):
    """Exact clipped-border bilinear for pixels whose flow leaves the device
    tap window.  Mirrors reference.py's math bit-for-bit in fp32."""
    f32 = np.float32
    flow = np.asarray(input2, dtype=f32).reshape(2, H, W)
    mask = (np.abs(flow[0]) > HALO) | (np.abs(flow[1]) > HALO)
    if not mask.any():
        return out
    img = np.asarray(input1, dtype=f32).reshape(H, W)
    pad = np.zeros((H + 2, W + 2), dtype=f32)
    pad[1:-1, 1:-1] = img
    hy, wx = np.nonzero(mask)
    Hu = (flow[0, hy, wx] + hy.astype(f32)).astype(f32) + f32(1.0)
    Wu = (flow[1, hy, wx] + wx.astype(f32)).astype(f32) + f32(1.0)
    hf = np.floor(Hu).astype(np.int32)
    hc = hf + 1
    wf = np.floor(Wu).astype(np.int32)
    wc = wf + 1
    hfc, hcc = np.clip(hf, 0, H + 1), np.clip(hc, 0, H + 1)
    wfc, wcc = np.clip(wf, 0, W + 1), np.clip(wc, 0, W + 1)
    dH = (hcc.astype(f32) - Hu).astype(f32)
    dW = (wcc.astype(f32) - Wu).astype(f32)
    out[hy, wx] = (
        pad[hfc, wfc] * (dW * dH)
        + pad[hcc, wfc] * (dW * (f32(1.0) - dH))
        + pad[hfc, wcc] * ((f32(1.0) - dW) * dH)
        + pad[hcc, wcc] * ((f32(1.0) - dW) * (f32(1.0) - dH))
    )
    return out


def kernel(input1, input2):
    from concourse.bass_utils import run_bass_kernel_spmd

    in_maps = shard_inputs(input1, input2)
    key = (SH, R_CHUNK, HALO)
    if key not in _NC_CACHE:
        _NC_CACHE[key] = build_nc(sh=SH, r_chunk=R_CHUNK)
    nc = _NC_CACHE[key]

    last_err = None
    for attempt in range(3):
        try:
            res = run_bass_kernel_spmd(nc, in_maps, core_ids=list(range(NCORES)))
            break
        except Exception as e:  # transient device desync — retry
            last_err = e
            time.sleep(5.0 * (attempt + 1))
    else:
        raise last_err
    out = np.concatenate([r["out"] for r in res.results], axis=0).astype(np.float32)

    out = _patch_outliers(out, input1, input2)
    return out.reshape(1, 1, H, W)


# revision 55
# speedup vs baseline: 1.1061x; 1.0098x over previous
"""Dense bilinear spatial-transformer warp — gatherless 5-row tap window on
device + host patch for outlier flow.

Device math, exact whenever both flow components lie in [-2, 2] (for N(0,1)
flow that covers ~91% of pixels; the |flow|>2 outliers are patched on the
host with the exact clipped-border gather):

  horizontal, per dy row (telescoped interpolation — 4 difference taps+base):
      H[dy] = B[y+dy, x-2] + sum_{dx=-2..1} D[y+dy, x+dx] * clamp(fw-dx, 0, 1)
      with the difference band D[j, c] = B[j, c+1] - B[j, c] precomputed on
      the host and shipped as a second fp16 input plane
  vertical (tri-weight, taps at the window edge get weight exactly 0):
      out   = sum_{dy=-2..2} relu(1 - |fh - dy|) * H[dy]

Engine split per row-chunk: the Scalar engine builds the per-axis tri-weight
stacks (Abs + Relu activations), the DVE runs the fp16 (2x-mode) tap
multiply/accumulate passes for dy planes 0:4 plus the vertical contraction,
and the GPSIMD/Pool engine independently evaluates the dy=+2 plane, which
balances the two engines' rates.  All three engines plus the DMAs pipeline
across row-chunks (triple-buffered accumulators); the first chunks are
staggered small to shorten pipeline fill.  The image ships as one fp16
zero-padded plane (halo 2); flow ships as raw fp32 row-shards (the
reference's +1 mesh shift folds into the activation biases); the output
returns as fp16 and is upcast on the host.
"""

import time
from contextlib import ExitStack

import numpy as np

import bass_rust
import concourse.bacc as bacc
import concourse.mybir as mybir
import concourse.tile as tile

F32 = mybir.dt.float32
F16 = mybir.dt.float16

H = 4096
W = 4096
NCORES = 8
SH = H // NCORES          # 512 rows per core
HALO = 2                  # tap window [-HALO, HALO] per axis
NTAP = 2 * HALO + 1       # 5
PADW = W + 2 * HALO       # padded image width (4100)
NPART = 128
CPB = W // NPART          # 32 columns per partition
CPB_H = CPB + 2 * HALO    # 36 columns incl. halo
R_CHUNK = 48
SPL = NTAP - 1            # dy planes handled by the DVE (Pool gets the last)


def _band_src_ap(t, row0, r):
    off = row0 * PADW
    return bass_rust.AP(
        tensor=t.ap().tensor, offset=off,
        ap=[[CPB, NPART], [PADW, r + 2 * HALO], [1, CPB_H]],
    )


def _flat_src_ap(t, row0, r, sh_w):
    off = row0 * sh_w
    return bass_rust.AP(
        tensor=t.ap().tensor, offset=off,
        ap=[[CPB, NPART], [sh_w, r], [1, CPB]],
    )


def _stack_view(tile_, width, col_off, nplanes, r):
    """[128, nplanes(dy), r, CPB] view; dy plane j reads rows shifted by j,
    cols shifted by col_off, of a [128, rows, width] tile."""
    base = tile_[:]
    return bass_rust.AP(
        tensor=base.tensor,
        offset=base.offset + col_off,
        ap=[list(base.ap[0]), [width, nplanes], [width, r], [1, CPB]],
    )


def _bcast_planes(ap2d, nplanes):
    return bass_rust.AP(
        tensor=ap2d.tensor, offset=ap2d.offset,
        ap=[list(ap2d.ap[0]), [0, nplanes]] + [list(d) for d in ap2d.ap[1:]],
    )


def _sub(ap, lo, hi):
    """Slice the plane dimension (axis 1) of a 4d AP."""
    return bass_rust.AP(
        tensor=ap.tensor,
        offset=ap.offset + lo * ap.ap[1][0],
        ap=[list(ap.ap[0]), [ap.ap[1][0], hi - lo]]
        + [list(d) for d in ap.ap[2:]],
    )


def build_nc(sh=SH, r_chunk=R_CHUNK, debug=False, head=(14, 18),
             tail=(24, 16, 8),
             out2_pool=False, out3_pool=True, stk_bufs=2, dma_merge=False,
             io_bufs=2, w_bufs=2, tele=True, vert_at=1):
    nc = bacc.Bacc("TRN2", target_bir_lowering=False, debug=debug)
    # stagger small chunks at both ends to shorten pipeline fill and drain
    head, tail = list(head), list(tail)
    body = (sh - sum(head) - sum(tail)) // r_chunk
    assert sum(head) + sum(tail) + body * r_chunk == sh
    chunks = []
    row0 = 0
    for r in head + [r_chunk] * body + tail:
        chunks.append((row0, r))
        row0 += r

    for v in range(-HALO, HALO + 1):
        val = float(v)
        if (F32, val) not in nc.const_aps.aps:
            t = nc.alloc_sbuf_tensor(f"const-float32-{val}", [128, 1], F32)
            nc.gpsimd.memset(t.ap(), val)
            nc.const_aps.aps[(F32, val)] = t.ap()
    nc.all_engine_barrier()

    img = nc.dram_tensor("img", [sh + 2 * HALO, PADW], F16, kind="ExternalInput")
    dimg = nc.dram_tensor("dimg", [sh + 2 * HALO, PADW - 1], F16,
                          kind="ExternalInput")
    fh = nc.dram_tensor("fh", [sh, W], F32, kind="ExternalInput")
    fw = nc.dram_tensor("fw", [sh, W], F32, kind="ExternalInput")
    out = nc.dram_tensor("out", [sh, W], F16, kind="ExternalOutput")

    ABS = mybir.ActivationFunctionType.Abs
    RELU = mybir.ActivationFunctionType.Relu
    MULT = mybir.AluOpType.mult
    ADD = mybir.AluOpType.add

    # (TensorScalarPtr is not a legal Pool-engine opcode on TRN2 silicon,
    # so the GPSIMD side sticks to plain tensor_tensor.)
    def pool_mul(out_ap, a, b):
        nc.gpsimd.tensor_mul(out_ap, a, b)

    def pool_add(out_ap, a, b):
        nc.gpsimd.tensor_add(out_ap, a, b)

    with tile.TileContext(nc) as tc, ExitStack() as ctx:
        io_pool = ctx.enter_context(tc.tile_pool(name="io", bufs=io_bufs))
        w_pool = ctx.enter_context(tc.tile_pool(name="wts", bufs=w_bufs))
        s_pool = ctx.enter_context(tc.tile_pool(name="stk", bufs=stk_bufs))
        o_pool = ctx.enter_context(tc.tile_pool(name="out", bufs=2))

        pending = [None]

        def emit_vert(row0, r, acc_a, pacc, astk):
            # vertical contraction for an earlier chunk (deferred so the
            # accumulator-merge DMA latency hides behind the next chunk's
            # tap passes when dma_merge is on)
            nc.vector.tensor_mul(acc_a[:], astk[:, :SPL], acc_a[:])
            pool_mul(pacc[:], astk[:, SPL:], pacc[:])
            out_t = o_pool.tile([NPART, r, CPB], F16, tag="out")
            nc.vector.tensor_add(acc_a[:, :2], acc_a[:, :2], acc_a[:, 2:4])
            if out2_pool:
                pool_add(out_t[:], acc_a[:, 0], acc_a[:, 1])
            else:
                nc.vector.tensor_add(out_t[:], acc_a[:, 0], acc_a[:, 1])
            if out3_pool:
                pool_add(out_t[:], out_t[:], pacc[:, 0])
            else:
                nc.vector.tensor_add(out_t[:], out_t[:], pacc[:, 0])
            nc.sync.dma_start(_flat_src_ap(out, row0, r, W), out_t[:])

        for row0, r in chunks:
            band = io_pool.tile([NPART, r + 2 * HALO, CPB_H], F16, tag="band")
            nc.sync.dma_start(band[:], _band_src_ap(img, row0, r))
            fh_t = io_pool.tile([NPART, r, CPB], F32, tag="fh")
            nc.sync.dma_start(fh_t[:], _flat_src_ap(fh, row0, r, W))
            fw_t = io_pool.tile([NPART, r, CPB], F32, tag="fw")
            nc.sync.dma_start(fw_t[:], _flat_src_ap(fw, row0, r, W))

            if tele:
                # telescoped horizontal: difference band D[j,c] = B[j,c+1] -
                # B[j,c] (precomputed on the host, DMA'd like the band) plus
                # clamp weights c(dx) = clamp(fw - dx, 0, 1):
                #   H = B[x-2] + sum_{dx=-2..1} D[x+dx] * c(dx)
                # (exact piecewise-linear interpolation for fw in [-2, 2];
                # one fewer tap pass per engine than the tri-weight form)
                dband = io_pool.tile([NPART, r + 2 * HALO, CPB_H - 1], F16,
                                     tag="dband")
                off = row0 * (PADW - 1)
                nc.sync.dma_start(dband[:], bass_rust.AP(
                    tensor=dimg.ap().tensor, offset=off,
                    ap=[[CPB, NPART], [PADW - 1, r + 2 * HALO],
                        [1, CPB_H - 1]]))

                bstk = w_pool.tile([NPART, NTAP - 1, r, CPB], F16, tag="bstk")
                for i, dx in enumerate(range(-HALO, HALO)):
                    nc.scalar.activation(bstk[:, i], fw_t[:], RELU,
                                         bias=float(-dx), scale=1.0)
                # clamp upper bound; tensor_scalar runs in DVE 4x mode
                # top tap dx=+1 needs no upper clamp: fw <= 2 in-window means
                # relu(fw-1) <= 1 already (out-of-window pixels stay finite
                # and are host-patched)
                nc.vector.tensor_scalar_min(bstk[:, :NTAP - 2],
                                            bstk[:, :NTAP - 2], 1.0)
            else:
                # horizontal tri-weight stack relu(1 - |fw - dx|), dx=-2..2
                bstk = w_pool.tile([NPART, NTAP, r, CPB], F16, tag="bstk")
                for i, dx in enumerate(range(-HALO, HALO + 1)):
                    nc.scalar.activation(bstk[:, i], fw_t[:], ABS,
                                         bias=float(-dx), scale=1.0)
                nc.scalar.activation(bstk[:], bstk[:], RELU,
                                     bias=1.0, scale=-1.0)

            # vertical tri-weight stack relu(1 - |fh - dy|), dy=-2..2
            astk = w_pool.tile([NPART, NTAP, r, CPB], F16, tag="astk")
            for i, dy in enumerate(range(-HALO, HALO + 1)):
                nc.scalar.activation(astk[:, i], fh_t[:], ABS,
                                     bias=float(-dy), scale=1.0)
            nc.scalar.activation(astk[:], astk[:], RELU, bias=1.0, scale=-1.0)

            acc_a = s_pool.tile([NPART, SPL, r, CPB], F16, tag="acc_a")
            tmp = s_pool.tile([NPART, SPL, r, CPB], F16, tag="tmp")
            pacc = s_pool.tile([NPART, 1, r, CPB], F16, tag="pacc")
            ptmp = s_pool.tile([NPART, 1, r, CPB], F16, tag="ptmp")

            if tele:
                ntx = NTAP - 1
                bviews = [_stack_view(dband, CPB_H - 1, dx + HALO, NTAP, r)
                          for dx in range(-HALO, HALO)]
                basev = _stack_view(band, CPB_H, 0, NTAP, r)
            else:
                ntx = NTAP
                bviews = [_stack_view(band, CPB_H, dx + HALO, NTAP, r)
                          for dx in range(-HALO, HALO + 1)]
                basev = None
            cviews = [_bcast_planes(bstk[:, i], NTAP) for i in range(ntx)]

            # DVE: dy planes 0:4
            nc.vector.tensor_mul(acc_a[:], _sub(cviews[0], 0, SPL),
                                 _sub(bviews[0], 0, SPL))
            for i in range(1, ntx):
                nc.vector.tensor_mul(tmp[:], _sub(cviews[i], 0, SPL),
                                     _sub(bviews[i], 0, SPL))
                nc.vector.tensor_add(acc_a[:], acc_a[:], tmp[:])
                if i == vert_at and pending[0] is not None:
                    emit_vert(*pending[0])
                    pending[0] = None
            if tele:
                nc.vector.tensor_add(acc_a[:], acc_a[:], _sub(basev, 0, SPL))

            # Pool: dy plane 4
            pool_mul(pacc[:], _sub(cviews[0], SPL, NTAP),
                     _sub(bviews[0], SPL, NTAP))
            for i in range(1, ntx):
                pool_mul(ptmp[:], _sub(cviews[i], SPL, NTAP),
                         _sub(bviews[i], SPL, NTAP))
                pool_add(pacc[:], pacc[:], ptmp[:])
            if tele:
                pool_add(pacc[:], pacc[:], _sub(basev, SPL, NTAP))

            pending[0] = (row0, r, acc_a, pacc, astk)

        emit_vert(*pending[0])

    nc.compile()
    return nc


def shard_inputs(input1, input2, sh=SH):
    img = np.asarray(input1, dtype=np.float32).reshape(H, W)
    flow = np.asarray(input2, dtype=np.float32).reshape(2, H, W)
    ncores = H // sh

    img_pad = np.zeros((H + 2 * HALO, PADW), dtype=np.float16)
    img_pad[HALO:H + HALO, HALO:W + HALO] = img

    # horizontal difference band of the padded image (fp16), incl. the
    # pad-boundary columns where one side is zero
    dimg_pad = np.zeros((H + 2 * HALO, PADW - 1), dtype=np.float16)
    dimg_pad[HALO:H + HALO, HALO:W + HALO - 1] = img[:, 1:] - img[:, :-1]
    dimg_pad[HALO:H + HALO, HALO - 1] = img[:, 0]
    dimg_pad[HALO:H + HALO, W + HALO - 1] = -img[:, W - 1]

    in_maps = []
    for k in range(ncores):
        h0 = k * sh
        in_maps.append({
            "img": np.ascontiguousarray(img_pad[h0:h0 + sh + 2 * HALO]),
            "dimg": np.ascontiguousarray(dimg_pad[h0:h0 + sh + 2 * HALO]),
            "fh": np.ascontiguousarray(flow[0, h0:h0 + sh]),
            "fw": np.ascontiguousarray(flow[1, h0:h0 + sh]),
        })
    return in_maps


_NC_CACHE = {}


def _patch_outliers(out, input1, input2):
    """Exact clipped-border bilinear for pixels whose flow leaves the device
    tap window.  Mirrors reference.py's math bit-for-bit in fp32."""
    f32 = np.float32
    flow = np.asarray(input2, dtype=f32).reshape(2, H, W)
    mask = (np.abs(flow[0]) > HALO) | (np.abs(flow[1]) > HALO)
    if not mask.any():
        return out
    img = np.asarray(input1, dtype=f32).reshape(H, W)
    pad = np.zeros((H + 2, W + 2), dtype=f32)
    pad[1:-1, 1:-1] = img
    hy, wx = np.nonzero(mask)
    Hu = (flow[0, hy, wx] + hy.astype(f32)).astype(f32) + f32(1.0)
    Wu = (flow[1, hy, wx] + wx.astype(f32)).astype(f32) + f32(1.0)
    hf = np.floor(Hu).astype(np.int32)
    hc = hf + 1
    wf = np.floor(Wu).astype(np.int32)
    wc = wf + 1
    hfc, hcc = np.clip(hf, 0, H + 1), np.clip(hc, 0, H + 1)
    wfc, wcc = np.clip(wf, 0, W + 1), np.clip(wc, 0, W + 1)
    dH = (hcc.astype(f32) - Hu).astype(f32)
    dW = (wcc.astype(f32) - Wu).astype(f32)
    out[hy, wx] = (
        pad[hfc, wfc] * (dW * dH)
        + pad[hcc, wfc] * (dW * (f32(1.0) - dH))
        + pad[hfc, wcc] * ((f32(1.0) - dW) * dH)
        + pad[hcc, wcc] * ((f32(1.0) - dW) * (f32(1.0) - dH))
    )
    return out


def kernel(input1, input2):
    from concourse.bass_utils import run_bass_kernel_spmd

    in_maps = shard_inputs(input1, input2)
    key = (SH, R_CHUNK, HALO)
    if key not in _NC_CACHE:
        _NC_CACHE[key] = build_nc(sh=SH, r_chunk=R_CHUNK)
    nc = _NC_CACHE[key]

    last_err = None
    for attempt in range(3):
        try:
            res = run_bass_kernel_spmd(nc, in_maps, core_ids=list(range(NCORES)))
            break
        except Exception as e:  # transient device desync — retry
            last_err = e
            time.sleep(5.0 * (attempt + 1))
    else:
        raise last_err
    out = np.concatenate([r["out"] for r in res.results], axis=0).astype(np.float32)

    out = _patch_outliers(out, input1, input2)
    return out.reshape(1, 1, H, W)


# revision 56
# speedup vs baseline: 1.1064x; 1.0003x over previous
"""Dense bilinear spatial-transformer warp — gatherless 5-row tap window on
device + host patch for outlier flow.

Device math, exact whenever both flow components lie in [-2, 2] (for N(0,1)
flow that covers ~91% of pixels; the |flow|>2 outliers are patched on the
host with the exact clipped-border gather):

  horizontal, per dy row (telescoped interpolation — 4 difference taps+base):
      H[dy] = B[y+dy, x-2] + sum_{dx=-2..1} D[y+dy, x+dx] * clamp(fw-dx, 0, 1)
      with the difference band D[j, c] = B[j, c+1] - B[j, c] precomputed on
      the host and shipped as a second fp16 input plane
  vertical (tri-weight, taps at the window edge get weight exactly 0):
      out   = sum_{dy=-2..2} relu(1 - |fh - dy|) * H[dy]

Engine split per row-chunk: the Scalar engine builds the per-axis tri-weight
stacks (Abs + Relu activations), the DVE runs the fp16 (2x-mode) tap
multiply/accumulate passes for dy planes 0:4 plus the vertical contraction,
and the GPSIMD/Pool engine independently evaluates the dy=+2 plane, which
balances the two engines' rates.  All three engines plus the DMAs pipeline
across row-chunks (triple-buffered accumulators); the first chunks are
staggered small to shorten pipeline fill.  The image ships as one fp16
zero-padded plane (halo 2); flow ships as raw fp32 row-shards (the
reference's +1 mesh shift folds into the activation biases); the output
returns as fp16 and is upcast on the host.
"""

import time
from contextlib import ExitStack

import numpy as np

import bass_rust
import concourse.bacc as bacc
import concourse.mybir as mybir
import concourse.tile as tile

F32 = mybir.dt.float32
F16 = mybir.dt.float16

H = 4096
W = 4096
NCORES = 8
SH = H // NCORES          # 512 rows per core
HALO = 2                  # tap window [-HALO, HALO] per axis
NTAP = 2 * HALO + 1       # 5
PADW = W + 2 * HALO       # padded image width (4100)
NPART = 128
CPB = W // NPART          # 32 columns per partition
CPB_H = CPB + 2 * HALO    # 36 columns incl. halo
R_CHUNK = 48
SPL = NTAP - 1            # dy planes handled by the DVE (Pool gets the last)


def _band_src_ap(t, row0, r):
    off = row0 * PADW
    return bass_rust.AP(
        tensor=t.ap().tensor, offset=off,
        ap=[[CPB, NPART], [PADW, r + 2 * HALO], [1, CPB_H]],
    )


def _flat_src_ap(t, row0, r, sh_w):
    off = row0 * sh_w
    return bass_rust.AP(
        tensor=t.ap().tensor, offset=off,
        ap=[[CPB, NPART], [sh_w, r], [1, CPB]],
    )


def _stack_view(tile_, width, col_off, nplanes, r):
    """[128, nplanes(dy), r, CPB] view; dy plane j reads rows shifted by j,
    cols shifted by col_off, of a [128, rows, width] tile."""
    base = tile_[:]
    return bass_rust.AP(
        tensor=base.tensor,
        offset=base.offset + col_off,
        ap=[list(base.ap[0]), [width, nplanes], [width, r], [1, CPB]],
    )


def _bcast_planes(ap2d, nplanes):
    return bass_rust.AP(
        tensor=ap2d.tensor, offset=ap2d.offset,
        ap=[list(ap2d.ap[0]), [0, nplanes]] + [list(d) for d in ap2d.ap[1:]],
    )


def _sub(ap, lo, hi):
    """Slice the plane dimension (axis 1) of a 4d AP."""
    return bass_rust.AP(
        tensor=ap.tensor,
        offset=ap.offset + lo * ap.ap[1][0],
        ap=[list(ap.ap[0]), [ap.ap[1][0], hi - lo]]
        + [list(d) for d in ap.ap[2:]],
    )


def build_nc(sh=SH, r_chunk=R_CHUNK, debug=False, head=(8, 24),
             tail=(24, 16, 8),
             out2_pool=False, out3_pool=True, stk_bufs=2, dma_merge=False,
             io_bufs=2, w_bufs=2, tele=True, vert_at=1):
    nc = bacc.Bacc("TRN2", target_bir_lowering=False, debug=debug)
    # stagger small chunks at both ends to shorten pipeline fill and drain
    head, tail = list(head), list(tail)
    body = (sh - sum(head) - sum(tail)) // r_chunk
    assert sum(head) + sum(tail) + body * r_chunk == sh
    chunks = []
    row0 = 0
    for r in head + [r_chunk] * body + tail:
        chunks.append((row0, r))
        row0 += r

    for v in range(-HALO, HALO + 1):
        val = float(v)
        if (F32, val) not in nc.const_aps.aps:
            t = nc.alloc_sbuf_tensor(f"const-float32-{val}", [128, 1], F32)
            nc.gpsimd.memset(t.ap(), val)
            nc.const_aps.aps[(F32, val)] = t.ap()
    nc.all_engine_barrier()

    img = nc.dram_tensor("img", [sh + 2 * HALO, PADW], F16, kind="ExternalInput")
    dimg = nc.dram_tensor("dimg", [sh + 2 * HALO, PADW - 1], F16,
                          kind="ExternalInput")
    fh = nc.dram_tensor("fh", [sh, W], F32, kind="ExternalInput")
    fw = nc.dram_tensor("fw", [sh, W], F32, kind="ExternalInput")
    out = nc.dram_tensor("out", [sh, W], F16, kind="ExternalOutput")

    ABS = mybir.ActivationFunctionType.Abs
    RELU = mybir.ActivationFunctionType.Relu
    MULT = mybir.AluOpType.mult
    ADD = mybir.AluOpType.add

    # (TensorScalarPtr is not a legal Pool-engine opcode on TRN2 silicon,
    # so the GPSIMD side sticks to plain tensor_tensor.)
    def pool_mul(out_ap, a, b):
        nc.gpsimd.tensor_mul(out_ap, a, b)

    def pool_add(out_ap, a, b):
        nc.gpsimd.tensor_add(out_ap, a, b)

    with tile.TileContext(nc) as tc, ExitStack() as ctx:
        io_pool = ctx.enter_context(tc.tile_pool(name="io", bufs=io_bufs))
        w_pool = ctx.enter_context(tc.tile_pool(name="wts", bufs=w_bufs))
        s_pool = ctx.enter_context(tc.tile_pool(name="stk", bufs=stk_bufs))
        o_pool = ctx.enter_context(tc.tile_pool(name="out", bufs=2))

        pending = [None]

        def emit_vert(row0, r, acc_a, pacc, astk):
            # vertical contraction for an earlier chunk (deferred so the
            # accumulator-merge DMA latency hides behind the next chunk's
            # tap passes when dma_merge is on)
            nc.vector.tensor_mul(acc_a[:], astk[:, :SPL], acc_a[:])
            pool_mul(pacc[:], astk[:, SPL:], pacc[:])
            out_t = o_pool.tile([NPART, r, CPB], F16, tag="out")
            nc.vector.tensor_add(acc_a[:, :2], acc_a[:, :2], acc_a[:, 2:4])
            if out2_pool:
                pool_add(out_t[:], acc_a[:, 0], acc_a[:, 1])
            else:
                nc.vector.tensor_add(out_t[:], acc_a[:, 0], acc_a[:, 1])
            if out3_pool:
                pool_add(out_t[:], out_t[:], pacc[:, 0])
            else:
                nc.vector.tensor_add(out_t[:], out_t[:], pacc[:, 0])
            nc.sync.dma_start(_flat_src_ap(out, row0, r, W), out_t[:])

        for row0, r in chunks:
            band = io_pool.tile([NPART, r + 2 * HALO, CPB_H], F16, tag="band")
            nc.sync.dma_start(band[:], _band_src_ap(img, row0, r))
            fh_t = io_pool.tile([NPART, r, CPB], F32, tag="fh")
            nc.sync.dma_start(fh_t[:], _flat_src_ap(fh, row0, r, W))
            fw_t = io_pool.tile([NPART, r, CPB], F32, tag="fw")
            nc.sync.dma_start(fw_t[:], _flat_src_ap(fw, row0, r, W))

            if tele:
                # telescoped horizontal: difference band D[j,c] = B[j,c+1] -
                # B[j,c] (precomputed on the host, DMA'd like the band) plus
                # clamp weights c(dx) = clamp(fw - dx, 0, 1):
                #   H = B[x-2] + sum_{dx=-2..1} D[x+dx] * c(dx)
                # (exact piecewise-linear interpolation for fw in [-2, 2];
                # one fewer tap pass per engine than the tri-weight form)
                dband = io_pool.tile([NPART, r + 2 * HALO, CPB_H - 1], F16,
                                     tag="dband")
                off = row0 * (PADW - 1)
                nc.sync.dma_start(dband[:], bass_rust.AP(
                    tensor=dimg.ap().tensor, offset=off,
                    ap=[[CPB, NPART], [PADW - 1, r + 2 * HALO],
                        [1, CPB_H - 1]]))

                bstk = w_pool.tile([NPART, NTAP - 1, r, CPB], F16, tag="bstk")
                for i, dx in enumerate(range(-HALO, HALO)):
                    nc.scalar.activation(bstk[:, i], fw_t[:], RELU,
                                         bias=float(-dx), scale=1.0)
                # clamp upper bound; tensor_scalar runs in DVE 4x mode
                # top tap dx=+1 needs no upper clamp: fw <= 2 in-window means
                # relu(fw-1) <= 1 already (out-of-window pixels stay finite
                # and are host-patched)
                nc.vector.tensor_scalar_min(bstk[:, :NTAP - 2],
                                            bstk[:, :NTAP - 2], 1.0)
            else:
                # horizontal tri-weight stack relu(1 - |fw - dx|), dx=-2..2
                bstk = w_pool.tile([NPART, NTAP, r, CPB], F16, tag="bstk")
                for i, dx in enumerate(range(-HALO, HALO + 1)):
                    nc.scalar.activation(bstk[:, i], fw_t[:], ABS,
                                         bias=float(-dx), scale=1.0)
                nc.scalar.activation(bstk[:], bstk[:], RELU,
                                     bias=1.0, scale=-1.0)

            # vertical tri-weight stack relu(1 - |fh - dy|), dy=-2..2
            astk = w_pool.tile([NPART, NTAP, r, CPB], F16, tag="astk")
            for i, dy in enumerate(range(-HALO, HALO + 1)):
                nc.scalar.activation(astk[:, i], fh_t[:], ABS,
                                     bias=float(-dy), scale=1.0)
            nc.scalar.activation(astk[:], astk[:], RELU, bias=1.0, scale=-1.0)

            acc_a = s_pool.tile([NPART, SPL, r, CPB], F16, tag="acc_a")
            tmp = s_pool.tile([NPART, SPL, r, CPB], F16, tag="tmp")
            pacc = s_pool.tile([NPART, 1, r, CPB], F16, tag="pacc")
            ptmp = s_pool.tile([NPART, 1, r, CPB], F16, tag="ptmp")

            if tele:
                ntx = NTAP - 1
                bviews = [_stack_view(dband, CPB_H - 1, dx + HALO, NTAP, r)
                          for dx in range(-HALO, HALO)]
                basev = _stack_view(band, CPB_H, 0, NTAP, r)
            else:
                ntx = NTAP
                bviews = [_stack_view(band, CPB_H, dx + HALO, NTAP, r)
                          for dx in range(-HALO, HALO + 1)]
                basev = None
            cviews = [_bcast_planes(bstk[:, i], NTAP) for i in range(ntx)]

            # DVE: dy planes 0:4
            nc.vector.tensor_mul(acc_a[:], _sub(cviews[0], 0, SPL),
                                 _sub(bviews[0], 0, SPL))
            for i in range(1, ntx):
                nc.vector.tensor_mul(tmp[:], _sub(cviews[i], 0, SPL),
                                     _sub(bviews[i], 0, SPL))
                nc.vector.tensor_add(acc_a[:], acc_a[:], tmp[:])
                if i == vert_at and pending[0] is not None:
                    emit_vert(*pending[0])
                    pending[0] = None
            if tele:
                nc.vector.tensor_add(acc_a[:], acc_a[:], _sub(basev, 0, SPL))

            # Pool: dy plane 4
            pool_mul(pacc[:], _sub(cviews[0], SPL, NTAP),
                     _sub(bviews[0], SPL, NTAP))
            for i in range(1, ntx):
                pool_mul(ptmp[:], _sub(cviews[i], SPL, NTAP),
                         _sub(bviews[i], SPL, NTAP))
                pool_add(pacc[:], pacc[:], ptmp[:])
            if tele:
                pool_add(pacc[:], pacc[:], _sub(basev, SPL, NTAP))

            pending[0] = (row0, r, acc_a, pacc, astk)

        emit_vert(*pending[0])

    nc.compile()
    return nc


def shard_inputs(input1, input2, sh=SH):
    img = np.asarray(input1, dtype=np.float32).reshape(H, W)
    flow = np.asarray(input2, dtype=np.float32).reshape(2, H, W)
    ncores = H // sh

    img_pad = np.zeros((H + 2 * HALO, PADW), dtype=np.float16)
    img_pad[HALO:H + HALO, HALO:W + HALO] = img

    # horizontal difference band of the padded image (fp16), incl. the
    # pad-boundary columns where one side is zero
    dimg_pad = np.zeros((H + 2 * HALO, PADW - 1), dtype=np.float16)
    dimg_pad[HALO:H + HALO, HALO:W + HALO - 1] = img[:, 1:] - img[:, :-1]
    dimg_pad[HALO:H + HALO, HALO - 1] = img[:, 0]
    dimg_pad[HALO:H + HALO, W + HALO - 1] = -img[:, W - 1]

    in_maps = []
    for k in range(ncores):
        h0 = k * sh
        in_maps.append({
            "img": np.ascontiguousarray(img_pad[h0:h0 + sh + 2 * HALO]),
            "dimg": np.ascontiguousarray(dimg_pad[h0:h0 + sh + 2 * HALO]),
            "fh": np.ascontiguousarray(flow[0, h0:h0 + sh]),
            "fw": np.ascontiguousarray(flow[1, h0:h0 + sh]),
        })
    return in_maps


_NC_CACHE = {}


def _patch_outliers(out, input1, input2):
    """Exact clipped-border bilinear for pixels whose flow leaves the device
    tap window.  Mirrors reference.py's math bit-for-bit in fp32."""
    f32 = np.float32
    flow = np.asarray(input2, dtype=f32).reshape(2, H, W)
    mask = (np.abs(flow[0]) > HALO) | (np.abs(flow[1]) > HALO)
    if not mask.any():
        return out
    img = np.asarray(input1, dtype=f32).reshape(H, W)
    pad = np.zeros((H + 2, W + 2), dtype=f32)
    pad[1:-1, 1:-1] = img
    hy, wx = np.nonzero(mask)
    Hu = (flow[0, hy, wx] + hy.astype(f32)).astype(f32) + f32(1.0)
    Wu = (flow[1, hy, wx] + wx.astype(f32)).astype(f32) + f32(1.0)
    hf = np.floor(Hu).astype(np.int32)
    hc = hf + 1
    wf = np.floor(Wu).astype(np.int32)
    wc = wf + 1
    hfc, hcc = np.clip(hf, 0, H + 1), np.clip(hc, 0, H + 1)
    wfc, wcc = np.clip(wf, 0, W + 1), np.clip(wc, 0, W + 1)
    dH = (hcc.astype(f32) - Hu).astype(f32)
    dW = (wcc.astype(f32) - Wu).astype(f32)
    out[hy, wx] = (
        pad[hfc, wfc] * (dW * dH)
        + pad[hcc, wfc] * (dW * (f32(1.0) - dH))
        + pad[hfc, wcc] * ((f32(1.0) - dW) * dH)
        + pad[hcc, wcc] * ((f32(1.0) - dW) * (f32(1.0) - dH))
    )
    return out


def kernel(input1, input2):
    from concourse.bass_utils import run_bass_kernel_spmd

    in_maps = shard_inputs(input1, input2)
    key = (SH, R_CHUNK, HALO)
    if key not in _NC_CACHE:
        _NC_CACHE[key] = build_nc(sh=SH, r_chunk=R_CHUNK)
    nc = _NC_CACHE[key]

    last_err = None
    for attempt in range(3):
        try:
            res = run_bass_kernel_spmd(nc, in_maps, core_ids=list(range(NCORES)))
            break
        except Exception as e:  # transient device desync — retry
            last_err = e
            time.sleep(5.0 * (attempt + 1))
    else:
        raise last_err
    out = np.concatenate([r["out"] for r in res.results], axis=0).astype(np.float32)

    out = _patch_outliers(out, input1, input2)
    return out.reshape(1, 1, H, W)


# revision 59
# speedup vs baseline: 1.1078x; 1.0013x over previous
"""Dense bilinear spatial-transformer warp — gatherless 5-row tap window on
device + host patch for outlier flow.

Device math, exact whenever both flow components lie in [-2, 2] (for N(0,1)
flow that covers ~91% of pixels; the |flow|>2 outliers are patched on the
host with the exact clipped-border gather):

  horizontal, per dy row (telescoped interpolation — 4 difference taps+base):
      H[dy] = B[y+dy, x-2] + sum_{dx=-2..1} D[y+dy, x+dx] * clamp(fw-dx, 0, 1)
      with the difference band D[j, c] = B[j, c+1] - B[j, c] precomputed on
      the host and shipped as a second fp16 input plane
  vertical (tri-weight, taps at the window edge get weight exactly 0):
      out   = sum_{dy=-2..2} relu(1 - |fh - dy|) * H[dy]

Engine split per row-chunk: the Scalar engine builds the per-axis tri-weight
stacks (Abs + Relu activations), the DVE runs the fp16 (2x-mode) tap
multiply/accumulate passes for dy planes 0:4 plus the vertical contraction,
and the GPSIMD/Pool engine independently evaluates the dy=+2 plane, which
balances the two engines' rates.  All three engines plus the DMAs pipeline
across row-chunks (triple-buffered accumulators); the first chunks are
staggered small to shorten pipeline fill.  The image ships as one fp16
zero-padded plane (halo 2); flow ships as raw fp32 row-shards (the
reference's +1 mesh shift folds into the activation biases); the output
returns as fp16 and is upcast on the host.
"""

import time
from contextlib import ExitStack

import numpy as np

import bass_rust
import concourse.bacc as bacc
import concourse.mybir as mybir
import concourse.tile as tile

F32 = mybir.dt.float32
F16 = mybir.dt.float16

H = 4096
W = 4096
NCORES = 8
SH = H // NCORES          # 512 rows per core
HALO = 2                  # tap window [-HALO, HALO] per axis
NTAP = 2 * HALO + 1       # 5
PADW = W + 2 * HALO       # padded image width (4100)
NPART = 128
CPB = W // NPART          # 32 columns per partition
CPB_H = CPB + 2 * HALO    # 36 columns incl. halo
R_CHUNK = 48
SPL = NTAP - 1            # dy planes handled by the DVE (Pool gets the last)


def _band_src_ap(t, row0, r):
    off = row0 * PADW
    return bass_rust.AP(
        tensor=t.ap().tensor, offset=off,
        ap=[[CPB, NPART], [PADW, r + 2 * HALO], [1, CPB_H]],
    )


def _flat_src_ap(t, row0, r, sh_w):
    off = row0 * sh_w
    return bass_rust.AP(
        tensor=t.ap().tensor, offset=off,
        ap=[[CPB, NPART], [sh_w, r], [1, CPB]],
    )


def _stack_view(tile_, width, col_off, nplanes, r):
    """[128, nplanes(dy), r, CPB] view; dy plane j reads rows shifted by j,
    cols shifted by col_off, of a [128, rows, width] tile."""
    base = tile_[:]
    return bass_rust.AP(
        tensor=base.tensor,
        offset=base.offset + col_off,
        ap=[list(base.ap[0]), [width, nplanes], [width, r], [1, CPB]],
    )


def _bcast_planes(ap2d, nplanes):
    return bass_rust.AP(
        tensor=ap2d.tensor, offset=ap2d.offset,
        ap=[list(ap2d.ap[0]), [0, nplanes]] + [list(d) for d in ap2d.ap[1:]],
    )


def _sub(ap, lo, hi):
    """Slice the plane dimension (axis 1) of a 4d AP."""
    return bass_rust.AP(
        tensor=ap.tensor,
        offset=ap.offset + lo * ap.ap[1][0],
        ap=[list(ap.ap[0]), [ap.ap[1][0], hi - lo]]
        + [list(d) for d in ap.ap[2:]],
    )


def build_nc(sh=SH, r_chunk=R_CHUNK, debug=False, head=(8, 24),
             tail=(32, 12, 4),
             out2_pool=False, out3_pool=True, stk_bufs=2, dma_merge=False,
             io_bufs=2, w_bufs=2, tele=True, vert_at=1):
    nc = bacc.Bacc("TRN2", target_bir_lowering=False, debug=debug)
    # stagger small chunks at both ends to shorten pipeline fill and drain
    head, tail = list(head), list(tail)
    body = (sh - sum(head) - sum(tail)) // r_chunk
    assert sum(head) + sum(tail) + body * r_chunk == sh
    chunks = []
    row0 = 0
    for r in head + [r_chunk] * body + tail:
        chunks.append((row0, r))
        row0 += r

    for v in range(-HALO, HALO + 1):
        val = float(v)
        if (F32, val) not in nc.const_aps.aps:
            t = nc.alloc_sbuf_tensor(f"const-float32-{val}", [128, 1], F32)
            nc.gpsimd.memset(t.ap(), val)
            nc.const_aps.aps[(F32, val)] = t.ap()
    nc.all_engine_barrier()

    img = nc.dram_tensor("img", [sh + 2 * HALO, PADW], F16, kind="ExternalInput")
    dimg = nc.dram_tensor("dimg", [sh + 2 * HALO, PADW - 1], F16,
                          kind="ExternalInput")
    fh = nc.dram_tensor("fh", [sh, W], F32, kind="ExternalInput")
    fw = nc.dram_tensor("fw", [sh, W], F32, kind="ExternalInput")
    out = nc.dram_tensor("out", [sh, W], F16, kind="ExternalOutput")

    ABS = mybir.ActivationFunctionType.Abs
    RELU = mybir.ActivationFunctionType.Relu
    MULT = mybir.AluOpType.mult
    ADD = mybir.AluOpType.add

    # (TensorScalarPtr is not a legal Pool-engine opcode on TRN2 silicon,
    # so the GPSIMD side sticks to plain tensor_tensor.)
    def pool_mul(out_ap, a, b):
        nc.gpsimd.tensor_mul(out_ap, a, b)

    def pool_add(out_ap, a, b):
        nc.gpsimd.tensor_add(out_ap, a, b)

    with tile.TileContext(nc) as tc, ExitStack() as ctx:
        io_pool = ctx.enter_context(tc.tile_pool(name="io", bufs=io_bufs))
        w_pool = ctx.enter_context(tc.tile_pool(name="wts", bufs=w_bufs))
        s_pool = ctx.enter_context(tc.tile_pool(name="stk", bufs=stk_bufs))
        o_pool = ctx.enter_context(tc.tile_pool(name="out", bufs=2))

        pending = [None]

        def emit_vert(row0, r, acc_a, pacc, astk):
            # vertical contraction for an earlier chunk (deferred so the
            # accumulator-merge DMA latency hides behind the next chunk's
            # tap passes when dma_merge is on)
            nc.vector.tensor_mul(acc_a[:], astk[:, :SPL], acc_a[:])
            pool_mul(pacc[:], astk[:, SPL:], pacc[:])
            out_t = o_pool.tile([NPART, r, CPB], F16, tag="out")
            nc.vector.tensor_add(acc_a[:, :2], acc_a[:, :2], acc_a[:, 2:4])
            if out2_pool:
                pool_add(out_t[:], acc_a[:, 0], acc_a[:, 1])
            else:
                nc.vector.tensor_add(out_t[:], acc_a[:, 0], acc_a[:, 1])
            if out3_pool:
                pool_add(out_t[:], out_t[:], pacc[:, 0])
            else:
                nc.vector.tensor_add(out_t[:], out_t[:], pacc[:, 0])
            nc.sync.dma_start(_flat_src_ap(out, row0, r, W), out_t[:])

        for row0, r in chunks:
            band = io_pool.tile([NPART, r + 2 * HALO, CPB_H], F16, tag="band")
            nc.sync.dma_start(band[:], _band_src_ap(img, row0, r))
            fh_t = io_pool.tile([NPART, r, CPB], F32, tag="fh")
            nc.sync.dma_start(fh_t[:], _flat_src_ap(fh, row0, r, W))
            fw_t = io_pool.tile([NPART, r, CPB], F32, tag="fw")
            nc.sync.dma_start(fw_t[:], _flat_src_ap(fw, row0, r, W))

            if tele:
                # telescoped horizontal: difference band D[j,c] = B[j,c+1] -
                # B[j,c] (precomputed on the host, DMA'd like the band) plus
                # clamp weights c(dx) = clamp(fw - dx, 0, 1):
                #   H = B[x-2] + sum_{dx=-2..1} D[x+dx] * c(dx)
                # (exact piecewise-linear interpolation for fw in [-2, 2];
                # one fewer tap pass per engine than the tri-weight form)
                dband = io_pool.tile([NPART, r + 2 * HALO, CPB_H - 1], F16,
                                     tag="dband")
                off = row0 * (PADW - 1)
                nc.sync.dma_start(dband[:], bass_rust.AP(
                    tensor=dimg.ap().tensor, offset=off,
                    ap=[[CPB, NPART], [PADW - 1, r + 2 * HALO],
                        [1, CPB_H - 1]]))

                bstk = w_pool.tile([NPART, NTAP - 1, r, CPB], F16, tag="bstk")
                for i, dx in enumerate(range(-HALO, HALO)):
                    nc.scalar.activation(bstk[:, i], fw_t[:], RELU,
                                         bias=float(-dx), scale=1.0)
                # clamp upper bound; tensor_scalar runs in DVE 4x mode
                # top tap dx=+1 needs no upper clamp: fw <= 2 in-window means
                # relu(fw-1) <= 1 already (out-of-window pixels stay finite
                # and are host-patched)
                nc.vector.tensor_scalar_min(bstk[:, :NTAP - 2],
                                            bstk[:, :NTAP - 2], 1.0)
            else:
                # horizontal tri-weight stack relu(1 - |fw - dx|), dx=-2..2
                bstk = w_pool.tile([NPART, NTAP, r, CPB], F16, tag="bstk")
                for i, dx in enumerate(range(-HALO, HALO + 1)):
                    nc.scalar.activation(bstk[:, i], fw_t[:], ABS,
                                         bias=float(-dx), scale=1.0)
                nc.scalar.activation(bstk[:], bstk[:], RELU,
                                     bias=1.0, scale=-1.0)

            # vertical tri-weight stack relu(1 - |fh - dy|), dy=-2..2
            astk = w_pool.tile([NPART, NTAP, r, CPB], F16, tag="astk")
            for i, dy in enumerate(range(-HALO, HALO + 1)):
                nc.scalar.activation(astk[:, i], fh_t[:], ABS,
                                     bias=float(-dy), scale=1.0)
            nc.scalar.activation(astk[:], astk[:], RELU, bias=1.0, scale=-1.0)

            acc_a = s_pool.tile([NPART, SPL, r, CPB], F16, tag="acc_a")
            tmp = s_pool.tile([NPART, SPL, r, CPB], F16, tag="tmp")
            pacc = s_pool.tile([NPART, 1, r, CPB], F16, tag="pacc")
            ptmp = s_pool.tile([NPART, 1, r, CPB], F16, tag="ptmp")

            if tele:
                ntx = NTAP - 1
                bviews = [_stack_view(dband, CPB_H - 1, dx + HALO, NTAP, r)
                          for dx in range(-HALO, HALO)]
                basev = _stack_view(band, CPB_H, 0, NTAP, r)
            else:
                ntx = NTAP
                bviews = [_stack_view(band, CPB_H, dx + HALO, NTAP, r)
                          for dx in range(-HALO, HALO + 1)]
                basev = None
            cviews = [_bcast_planes(bstk[:, i], NTAP) for i in range(ntx)]

            # DVE: dy planes 0:4
            nc.vector.tensor_mul(acc_a[:], _sub(cviews[0], 0, SPL),
                                 _sub(bviews[0], 0, SPL))
            for i in range(1, ntx):
                nc.vector.tensor_mul(tmp[:], _sub(cviews[i], 0, SPL),
                                     _sub(bviews[i], 0, SPL))
                nc.vector.tensor_add(acc_a[:], acc_a[:], tmp[:])
                if i == vert_at and pending[0] is not None:
                    emit_vert(*pending[0])
                    pending[0] = None
            if tele:
                nc.vector.tensor_add(acc_a[:], acc_a[:], _sub(basev, 0, SPL))

            # Pool: dy plane 4
            pool_mul(pacc[:], _sub(cviews[0], SPL, NTAP),
                     _sub(bviews[0], SPL, NTAP))
            for i in range(1, ntx):
                pool_mul(ptmp[:], _sub(cviews[i], SPL, NTAP),
                         _sub(bviews[i], SPL, NTAP))
                pool_add(pacc[:], pacc[:], ptmp[:])
            if tele:
                pool_add(pacc[:], pacc[:], _sub(basev, SPL, NTAP))

            pending[0] = (row0, r, acc_a, pacc, astk)

        emit_vert(*pending[0])

    nc.compile()
    return nc


def shard_inputs(input1, input2, sh=SH):
    img = np.asarray(input1, dtype=np.float32).reshape(H, W)
    flow = np.asarray(input2, dtype=np.float32).reshape(2, H, W)
    ncores = H // sh

    img_pad = np.zeros((H + 2 * HALO, PADW), dtype=np.float16)
    img_pad[HALO:H + HALO, HALO:W + HALO] = img

    # horizontal difference band of the padded image (fp16), incl. the
    # pad-boundary columns where one side is zero
    dimg_pad = np.zeros((H + 2 * HALO, PADW - 1), dtype=np.float16)
    dimg_pad[HALO:H + HALO, HALO:W + HALO - 1] = img[:, 1:] - img[:, :-1]
    dimg_pad[HALO:H + HALO, HALO - 1] = img[:, 0]
    dimg_pad[HALO:H + HALO, W + HALO - 1] = -img[:, W - 1]

    in_maps = []
    for k in range(ncores):
        h0 = k * sh
        in_maps.append({
            "img": np.ascontiguousarray(img_pad[h0:h0 + sh + 2 * HALO]),
            "dimg": np.ascontiguousarray(dimg_pad[h0:h0 + sh + 2 * HALO]),
            "fh": np.ascontiguousarray(flow[0, h0:h0 + sh]),
            "fw": np.ascontiguousarray(flow[1, h0:h0 + sh]),
        })
    return in_maps


_NC_CACHE = {}


def _patch_outliers(out, input1, input2):
    """Exact clipped-border bilinear for pixels whose flow leaves the device
    tap window.  Mirrors reference.py's math bit-for-bit in fp32."""
    f32 = np.float32
    flow = np.asarray(input2, dtype=f32).reshape(2, H, W)
    mask = (np.abs(flow[0]) > HALO) | (np.abs(flow[1]) > HALO)
    if not mask.any():
        return out
    img = np.asarray(input1, dtype=f32).reshape(H, W)
    pad = np.zeros((H + 2, W + 2), dtype=f32)
    pad[1:-1, 1:-1] = img
    hy, wx = np.nonzero(mask)
    Hu = (flow[0, hy, wx] + hy.astype(f32)).astype(f32) + f32(1.0)
    Wu = (flow[1, hy, wx] + wx.astype(f32)).astype(f32) + f32(1.0)
    hf = np.floor(Hu).astype(np.int32)
    hc = hf + 1
    wf = np.floor(Wu).astype(np.int32)
    wc = wf + 1
    hfc, hcc = np.clip(hf, 0, H + 1), np.clip(hc, 0, H + 1)
    wfc, wcc = np.clip(wf, 0, W + 1), np.clip(wc, 0, W + 1)
    dH = (hcc.astype(f32) - Hu).astype(f32)
    dW = (wcc.astype(f32) - Wu).astype(f32)
    out[hy, wx] = (
        pad[hfc, wfc] * (dW * dH)
        + pad[hcc, wfc] * (dW * (f32(1.0) - dH))
        + pad[hfc, wcc] * ((f32(1.0) - dW) * dH)
        + pad[hcc, wcc] * ((f32(1.0) - dW) * (f32(1.0) - dH))
    )
    return out


def kernel(input1, input2):
    from concourse.bass_utils import run_bass_kernel_spmd

    in_maps = shard_inputs(input1, input2)
    key = (SH, R_CHUNK, HALO)
    if key not in _NC_CACHE:
        _NC_CACHE[key] = build_nc(sh=SH, r_chunk=R_CHUNK)
    nc = _NC_CACHE[key]

    last_err = None
    for attempt in range(3):
        try:
            res = run_bass_kernel_spmd(nc, in_maps, core_ids=list(range(NCORES)))
            break
        except Exception as e:  # transient device desync — retry
            last_err = e
            time.sleep(5.0 * (attempt + 1))
    else:
        raise last_err
    out = np.concatenate([r["out"] for r in res.results], axis=0).astype(np.float32)

    out = _patch_outliers(out, input1, input2)
    return out.reshape(1, 1, H, W)
